# revision 1
# baseline (speedup 1.0000x reference)
"""Trainium2 Bass kernel for ContinuousConv1DSim (gnn_message_passing).

Reformulation (validated vs reference in fp32 numpy, rel err ~4e-5):
  G = F * npm (per-l mask), H = G * t
  MM1  (PE): psw[c2, l] = sum_j GH[j, c2] * Band[j, l]   -- causal 8-wide window
             sums over l, output TRANSPOSED (channels on partitions), with a
             second accumulating matmul adding the previous tile's halo rows.
  MM2a (PE): psp[l, 0:64]  = A_e   (window(G) @ W^T)
             psp[l, 64:128]= D_raw (window(H) @ W^T - window(G) @ bias)
  MM2b (PE): pssp[l, s*64+o] = u[s] * A_e[l, o]          -- s-expansion on PE
  sim_m   = (A_m * t - D_m) with A_m/D_m = npm * psp     (ACT copy w/ scale)
  obuf_sim= pssp * udt + sim_m (broadcast over s)        -- one DVE STT
  real[l] = npm[l] * (t[l] * A_m[l-1] - D_m[l-1])        -- partition-shifted STT
Output rows per l: [real, sim + u_s * udt * A] for s=0..7, last row real[L-1].

Pure data parallel: batch 32 -> 8 cores x 4. All params replicated.
"""

import numpy as np

B, L, C, O, S = 32, 2048, 64, 64, 8
NCORES = 8
BPC = B // NCORES          # 4 batches per core
NT = L // 128              # 16 l-tiles per batch
ROWS = (L - 1) * (S + 1) + 1  # 18424
F32 = None  # set after mybir import


def _consts(W, bias, u):
    n = np.arange(128)
    bandc = ((n[:, None] >= n[None, :] - 7) & (n[:, None] <= n[None, :])).astype(np.float32)
    bandp = (n[:, None] >= n[None, :] + 121).astype(np.float32)
    prba = np.zeros((128, 128), np.float32)
    prba[0:64, 0:64] = W.T           # A_e from U
    prba[0:64, 64:128] = -bias       # -F_e into D_raw
    prba[64:128, 64:128] = W.T       # TA_e into D_raw
    prbb = np.zeros((128, 512), np.float32)
    for s in range(S):
        prbb[0:64, s * 64:(s + 1) * 64] = u[s] * W.T
    return bandc, bandp, prba, prbb


def _build_nc():
    import concourse.bass as bass
    import concourse.bacc as bacc
    import concourse.mybir as mybir
    import concourse.tile as tile

    f32 = mybir.dt.float32
    Copy = mybir.ActivationFunctionType.Copy
    mult = mybir.AluOpType.mult
    sub = mybir.AluOpType.subtract
    add = mybir.AluOpType.add

    nc = bacc.Bacc("TRN2", target_bir_lowering=False, debug=False,
                   num_devices=NCORES)

    FD = nc.dram_tensor("f", [BPC, L, C], f32, kind="ExternalInput").ap()
    TSD = nc.dram_tensor("ts", [BPC, L + 128], f32, kind="ExternalInput").ap()
    UDD = nc.dram_tensor("ud", [BPC, L], f32, kind="ExternalInput").ap()
    NPD = nc.dram_tensor("np", [BPC, L + 128], f32, kind="ExternalInput").ap()
    BCD = nc.dram_tensor("bandc", [128, 128], f32, kind="ExternalInput").ap()
    BPD = nc.dram_tensor("bandp", [128, 128], f32, kind="ExternalInput").ap()
    PAD = nc.dram_tensor("prba", [128, 128], f32, kind="ExternalInput").ap()
    PBD = nc.dram_tensor("prbb", [128, 512], f32, kind="ExternalInput").ap()
    OUTD = nc.dram_tensor("out", [BPC, ROWS, O], f32, kind="ExternalOutput").ap()

    with tile.TileContext(nc) as tc:
        with (
            tc.tile_pool(name="const", bufs=1) as cpool,
            tc.tile_pool(name="scal", bufs=2) as spool,
            tc.tile_pool(name="feat", bufs=3) as fpool,
            tc.tile_pool(name="gh", bufs=3) as ghpool,
            tc.tile_pool(name="sbw", bufs=3) as sbwpool,
            tc.tile_pool(name="pp", bufs=3) as pppool,
            tc.tile_pool(name="simm", bufs=3) as simpool,
            tc.tile_pool(name="ob", bufs=3) as obpool,
            tc.tile_pool(name="ro", bufs=3) as ropool,
            tc.tile_pool(name="psw", bufs=3, space=bass.MemorySpace.PSUM) as pwpool,
            tc.tile_pool(name="psp", bufs=2, space=bass.MemorySpace.PSUM) as papool,
            tc.tile_pool(name="pssp", bufs=2, space=bass.MemorySpace.PSUM) as pbpool,
        ):
            bandc_t = cpool.tile([128, 128], f32, tag="bandc")
            bandp_t = cpool.tile([128, 128], f32, tag="bandp")
            prba_t = cpool.tile([128, 128], f32, tag="prba")
            prbb_t = cpool.tile([128, 512], f32, tag="prbb")
            zrow = cpool.tile([1, 64], f32, tag="zrow")
            nc.sync.dma_start(bandc_t[:], BCD)
            nc.sync.dma_start(bandp_t[:], BPD)
            nc.sync.dma_start(prba_t[:], PAD)
            nc.sync.dma_start(prbb_t[:], PBD)
            nc.gpsimd.memset(zrow[:], 0.0)

            for b in range(BPC):
                tst = spool.tile([128, NT], f32, tag="tst")
                tsh = spool.tile([128, NT], f32, tag="tsh")
                udt = spool.tile([128, NT], f32, tag="udt")
                npt = spool.tile([128, NT], f32, tag="npt")
                nsh = spool.tile([128, NT], f32, tag="nsh")
                nc.sync.dma_start(tst[:], TSD[b, 0:L].rearrange("(n p) -> p n", p=128))
                nc.sync.dma_start(tsh[:], TSD[b, 1:L + 1].rearrange("(n p) -> p n", p=128))
                nc.sync.dma_start(udt[:], UDD[b].rearrange("(n p) -> p n", p=128))
                nc.sync.dma_start(npt[:], NPD[b, 0:L].rearrange("(n p) -> p n", p=128))
                nc.sync.dma_start(nsh[:], NPD[b, 1:L + 1].rearrange("(n p) -> p n", p=128))
                # real row for l=0 is identically zero
                nc.sync.dma_start(OUTD[b, 0:1, :], zrow[:])

                psw_next = None
                for n in range(NT):
                    ftile = fpool.tile([128, C], f32, tag="f")
                    nc.sync.dma_start(ftile[:], FD[b, n * 128:(n + 1) * 128, :])
                    gh = ghpool.tile([128, 128], f32, tag="gh")
                    nc.scalar.activation(gh[:, 0:64], ftile[:], Copy,
                                         scale=npt[:, n:n + 1])
                    nc.vector.tensor_scalar_mul(gh[:, 64:128], gh[:, 0:64],
                                                tst[:, n:n + 1])
                    # MM1: windowed sums, transposed output
                    if n == 0:
                        psw_cur = pwpool.tile([128, 128], f32, tag="psw")
                        nc.tensor.matmul(psw_cur[:], gh[:], bandc_t[:],
                                         start=True, stop=True)
                    else:
                        psw_cur = psw_next
                        nc.tensor.matmul(psw_cur[:], gh[:], bandc_t[:],
                                         start=False, stop=True)
                    if n < NT - 1:
                        psw_next = pwpool.tile([128, 128], f32, tag="psw")
                        nc.tensor.matmul(psw_next[:], gh[:], bandp_t[:],
                                         start=True, stop=False)
                    sbw = sbwpool.tile([128, 128], f32, tag="sbw")
                    nc.scalar.copy(sbw[:], psw_cur[:])
                    # MM2: project windowed features
                    psp = papool.tile([128, 128], f32, tag="psp")
                    nc.tensor.matmul(psp[:], sbw[:], prba_t[:], start=True, stop=True)
                    pssp = pbpool.tile([128, 512], f32, tag="pssp")
                    nc.tensor.matmul(pssp[:], sbw[:], prbb_t[:], start=True, stop=True)
                    pp = pppool.tile([128, 128], f32, tag="pp")
                    nc.scalar.activation(pp[:], psp[:], Copy, scale=npt[:, n:n + 1])
                    sim_m = simpool.tile([128, 64], f32, tag="simm")
                    nc.vector.scalar_tensor_tensor(
                        sim_m[:], pp[:, 0:64], tst[:, n:n + 1], pp[:, 64:128],
                        op0=mult, op1=sub)
                    obsim = obpool.tile([128, 512], f32, tag="ob")
                    nc.vector.scalar_tensor_tensor(
                        obsim[:].rearrange("p (s o) -> p s o", o=64),
                        pssp[:].rearrange("p (s o) -> p s o", o=64),
                        udt[:, n:n + 1],
                        sim_m[:].unsqueeze(1).broadcast_to([128, 8, 64]),
                        op0=mult, op1=add)
                    # real rows for l = l0+1 .. l0+128, lane p -> l0+p+1
                    rr = ropool.tile([128, 64], f32, tag="rr")
                    nc.vector.scalar_tensor_tensor(
                        rr[:], pp[:, 0:64], tsh[:, n:n + 1], pp[:, 64:128],
                        op0=mult, op1=sub)
                    rm = ropool.tile([128, 64], f32, tag="rm")
                    nc.vector.tensor_scalar_mul(rm[:], rr[:], nsh[:, n:n + 1])
                    # store
                    PR = 128 if n < NT - 1 else 127
                    real_dst = bass.AP(
                        OUTD.tensor, (b * ROWS + 9 * (n * 128 + 1)) * 64,
                        [[9 * 64, PR], [1, 64]])
                    nc.sync.dma_start(real_dst, rm[0:PR, :])
                    if n < NT - 1:
                        blk = OUTD[b, 9 * n * 128: 9 * (n + 1) * 128, :] \
                            .rearrange("(p s) o -> p s o", s=9)
                        nc.sync.dma_start(blk[:, 1:9, :],
                                          obsim[:].rearrange("p (s o) -> p s o", o=64))
                    else:
                        blk = OUTD[b, 9 * n * 128: 9 * n * 128 + 9 * 127, :] \
                            .rearrange("(p s) o -> p s o", s=9)
                        nc.sync.dma_start(
                            blk[:, 1:9, :],
                            obsim[0:127, :].rearrange("p (s o) -> p s o", o=64))
    nc.compile()
    return nc


_NC_CACHE = None


def kernel(**inputs):
    global _NC_CACHE
    from concourse.bass_utils import run_bass_kernel_spmd

    times = np.ascontiguousarray(inputs["times"], np.float32)
    feats = np.ascontiguousarray(inputs["features"], np.float32)
    npm = inputs["non_pad_mask"].astype(np.float32)
    u = np.asarray(inputs["uniform_sample"], np.float32)
    W = np.ascontiguousarray(inputs["W"], np.float32)
    bias = np.ascontiguousarray(inputs["bias_param"], np.float32)

    bandc, bandp, prba, prbb = _consts(W, bias, u)
    tnext = np.concatenate([times[:, 1:], np.zeros((B, 1), np.float32)], 1)
    npmn = np.concatenate([npm[:, 1:], np.zeros((B, 1), np.float32)], 1)
    udt = (tnext - times) * npm * npmn  # (B, L); l=L-1 col unused downstream

    if _NC_CACHE is None:
        _NC_CACHE = _build_nc()
    nc = _NC_CACHE

    pad = np.zeros((B, 128), np.float32)
    times_p = np.concatenate([times, pad], 1)
    npm_p = np.concatenate([npm, pad], 1)

    in_maps = []
    for c in range(NCORES):
        sl = slice(c * BPC, (c + 1) * BPC)
        in_maps.append({
            "f": np.ascontiguousarray(feats[sl]),
            "ts": np.ascontiguousarray(times_p[sl]),
            "ud": np.ascontiguousarray(udt[sl]),
            "np": np.ascontiguousarray(npm_p[sl]),
            "bandc": bandc, "bandp": bandp, "prba": prba, "prbb": prbb,
        })
    res = run_bass_kernel_spmd(nc, in_maps, core_ids=list(range(NCORES)))
    out = np.concatenate([r["out"] for r in res.results], 0)
    return out.astype(np.float32)



# revision 4
# speedup vs baseline: 1.1907x; 1.1907x over previous
"""Trainium2 Bass kernel for ContinuousConv1DSim (gnn_message_passing).

Delta-band bf16 formulation (validated in numpy emulation, rel err ~2.9e-3):
  Overlapping l-tiles of 128 events at stride 121 (window Ks=9 fully inside
  a tile for lanes p>=7), so no halo accumulation chain.
  G = F * npm_j (pre-masked on host, bf16).
  Per batch, build three 8-diagonal band operands [j, (n,p)] on DVE from
  gpsimd partition-broadcast rows:
    Bm    = npm_l * band            (masks output column l)
    BandD = (tloc_l - tloc_j) * Bm  (delta-weighted band; tloc centered
                                     per tile so bf16 stays accurate)
    BandU = udt_l * band            (udt = masked dt to next event)
  MM1 (PE, bf16): ptile[c2, p] accumulates [SD | SG] and [SU]:
    SD = G^T @ BandD, SG = G^T @ Bm, SU = G^T @ BandU
  MM2 (PE, bf16): obsim[p, 576] = [SD|SG]^T @ prbA + [SU]^T @ prbB
    9 col-blocks of 64: s=0..7 sim rows and a u=1 block.
    block s: SD@W^T + SG@bias + udt*u_s*(SG@W^T)  == sim row s
    u1 block * nsh == real row for l+1 (right-padding makes npm monotone).
  Copies PSUM->SBUF split across ACT/DVE, output DMA'd as 2 KB packets.

Pure data parallel: batch 32 -> 8 cores x 4. All params replicated.
"""

import numpy as np

B, L, C, O, S = 32, 2048, 64, 64, 8
NCORES = 8
BPC = B // NCORES            # 4 batches per core
STRIDE = 121                 # l-tile stride (128 - 7 overlap)
NT2 = 17                     # tiles per batch: 121*16 + 128 > 2048
EXT = NT2 * 128              # 2176
ROWS = (L - 1) * (S + 1) + 1  # 18424


def _consts(W, bias, u):
    import ml_dtypes
    n = np.arange(128)
    band = ((n[:, None] >= n[None, :] - 7) & (n[:, None] <= n[None, :]))
    band = band.astype(np.float32)
    WT = W.T.astype(np.float32)
    prbA = np.zeros((128, 576), np.float32)
    prbB = np.zeros((64, 576), np.float32)
    for k in range(9):
        sl = slice(k * 64, (k + 1) * 64)
        prbA[0:64, sl] = WT           # SD rows -> W^T
        prbA[64:128, sl] = bias       # SG rows -> bias
        prbB[0:64, sl] = (u[k] if k < 8 else 1.0) * WT  # SU rows
    bf = ml_dtypes.bfloat16
    return band.astype(bf), prbA.astype(bf), prbB.astype(bf)


def _host_prep(times, feats, npm):
    """Per-full-batch host tensors (numpy, cheap)."""
    import ml_dtypes
    bf = ml_dtypes.bfloat16
    IDX = (STRIDE * np.arange(NT2))[None, :] + np.arange(128)[:, None]  # [p, n]
    IDXf = IDX.T.reshape(-1)        # [(n, p)] flattened

    G = feats * npm[:, :, None]     # mask padded events
    G_ext = np.concatenate([G, np.zeros((B, 256, C), np.float32)], 1)
    t_ext = np.concatenate([times, np.repeat(times[:, -1:], 256, 1)], 1)
    npm_ext = np.concatenate([npm, np.zeros((B, 256), np.float32)], 1)
    udt = np.zeros((B, L + 256), np.float32)
    udt[:, :L - 1] = (times[:, 1:] - times[:, :-1]) * npm[:, :-1] * npm[:, 1:]

    cen = times[:, STRIDE * np.arange(NT2)]          # [B, n]
    trow = (t_ext[:, IDXf].reshape(B, NT2, 128)
            - cen[:, :, None]).reshape(B, EXT).astype(np.float32)
    udtrow = udt[:, IDXf].astype(bf)                 # [B, EXT]
    npmrow = npm_ext[:, IDXf].astype(bf)
    # column scalars [p, n]: tloc and nsh (npm at l+1)
    scol = np.empty((B, 128, 2 * NT2), np.float32)
    scol[:, :, :NT2] = trow.reshape(B, NT2, 128).transpose(0, 2, 1)
    scol[:, :, NT2:] = npm_ext[:, IDX + 1]
    hostF = G_ext[:, IDX, :].reshape(B, 128, NT2 * C).astype(bf)  # [B,p,(n c)]
    return hostF, trow, udtrow, npmrow, scol


def _build_nc():
    import concourse.bass as bass
    import concourse.bacc as bacc
    import concourse.mybir as mybir
    import concourse.tile as tile

    f32 = mybir.dt.float32
    bf16 = mybir.dt.bfloat16
    Copy = mybir.ActivationFunctionType.Copy
    mult = mybir.AluOpType.mult
    add = mybir.AluOpType.add
    sub = mybir.AluOpType.subtract

    nc = bacc.Bacc("TRN2", target_bir_lowering=False, debug=False,
                   num_devices=NCORES)

    FD = nc.dram_tensor("f", [BPC, 128, NT2 * C], bf16, kind="ExternalInput").ap()
    TRD = nc.dram_tensor("trow", [BPC, EXT], f32, kind="ExternalInput").ap()
    URD = nc.dram_tensor("udtrow", [BPC, EXT], bf16, kind="ExternalInput").ap()
    NRD = nc.dram_tensor("npmrow", [BPC, EXT], bf16, kind="ExternalInput").ap()
    SCD = nc.dram_tensor("scol", [BPC, 128, 2 * NT2], f32, kind="ExternalInput").ap()
    BDD = nc.dram_tensor("band", [128, 128], bf16, kind="ExternalInput").ap()
    PAD = nc.dram_tensor("prbA", [128, 576], bf16, kind="ExternalInput").ap()
    PBD = nc.dram_tensor("prbB", [64, 576], bf16, kind="ExternalInput").ap()
    OUTD = nc.dram_tensor("out", [BPC, ROWS, O], f32, kind="ExternalOutput").ap()

    with tile.TileContext(nc) as tc:
        with (
            tc.tile_pool(name="const", bufs=1) as cpool,
            tc.tile_pool(name="rows", bufs=2) as rpool,
            tc.tile_pool(name="rep", bufs=2) as bpool,
            tc.tile_pool(name="bands", bufs=2) as dpool,
            tc.tile_pool(name="feat", bufs=2) as fpool,
            tc.tile_pool(name="sbw", bufs=3) as spool,
            tc.tile_pool(name="osb", bufs=3) as opool,
            tc.tile_pool(name="rsb", bufs=3) as lpool,
            tc.tile_pool(name="pt", bufs=3, space=bass.MemorySpace.PSUM) as ppool,
            tc.tile_pool(name="po", bufs=2, space=bass.MemorySpace.PSUM) as qpool,
            tc.tile_pool(name="po2", bufs=2, space=bass.MemorySpace.PSUM) as rpool2,
        ):
            band_t = cpool.tile([128, 128], bf16, tag="band")
            prbA_t = cpool.tile([128, 576], bf16, tag="prbA")
            prbB_t = cpool.tile([64, 576], bf16, tag="prbB")
            zrow = cpool.tile([1, 64], f32, tag="zrow")
            nc.sync.dma_start(band_t[:], BDD)
            nc.sync.dma_start(prbA_t[:], PAD)
            nc.sync.dma_start(prbB_t[:], PBD)
            nc.gpsimd.memset(zrow[:], 0.0)
            bandv = band_t[:].unsqueeze(1).broadcast_to([128, NT2, 128])

            for b in range(BPC):
                trow_t = rpool.tile([1, EXT], f32, tag="trow")
                urow_t = rpool.tile([1, EXT], bf16, tag="urow")
                nrow_t = rpool.tile([1, EXT], bf16, tag="nrow")
                scol_t = rpool.tile([128, 2 * NT2], f32, tag="scol")
                fsb = fpool.tile([128, NT2 * C], bf16, tag="f")
                nc.sync.dma_start(trow_t[:], TRD[b].unsqueeze(0))
                nc.sync.dma_start(urow_t[:], URD[b].unsqueeze(0))
                nc.sync.dma_start(nrow_t[:], NRD[b].unsqueeze(0))
                nc.sync.dma_start(scol_t[:], SCD[b])
                nc.sync.dma_start(fsb[:], FD[b])
                nc.sync.dma_start(OUTD[b, 0:1, :], zrow[:])  # real row l=0

                trep = bpool.tile([128, EXT], f32, tag="trep")
                urep = bpool.tile([128, EXT], bf16, tag="urep")
                nrep = bpool.tile([128, EXT], bf16, tag="nrep")
                nc.gpsimd.partition_broadcast(trep[:], trow_t[:])
                nc.gpsimd.partition_broadcast(urep[:], urow_t[:])
                nc.gpsimd.partition_broadcast(nrep[:], nrow_t[:])

                tcolv = scol_t[:, 0:NT2].unsqueeze(2).broadcast_to([128, NT2, 128])
                bmb = dpool.tile([128, EXT], bf16, tag="bm")
                dif = dpool.tile([128, EXT], bf16, tag="dif")
                bdd = dpool.tile([128, EXT], bf16, tag="bd")
                bud = dpool.tile([128, EXT], bf16, tag="bu")
                t3 = [128, NT2, 128]
                nc.vector.scalar_tensor_tensor(
                    bmb[:].rearrange("p (n l) -> p n l", l=128),
                    nrep[:].rearrange("p (n l) -> p n l", l=128), 1.0, bandv,
                    op0=mult, op1=mult)
                nc.vector.scalar_tensor_tensor(
                    dif[:].rearrange("p (n l) -> p n l", l=128),
                    trep[:].rearrange("p (n l) -> p n l", l=128), 0.0, tcolv,
                    op0=add, op1=sub)
                nc.vector.scalar_tensor_tensor(
                    bdd[:], dif[:], 1.0, bmb[:], op0=mult, op1=mult)
                nc.vector.scalar_tensor_tensor(
                    bud[:].rearrange("p (n l) -> p n l", l=128),
                    urep[:].rearrange("p (n l) -> p n l", l=128), 1.0, bandv,
                    op0=mult, op1=mult)

                for n in range(NT2):
                    ls = slice(n * 128, (n + 1) * 128)
                    G_n = fsb[:, n * C:(n + 1) * C]
                    ptile = ppool.tile([128, 256], f32, tag="pt")
                    # SD -> partitions 0:64, SG -> 64:128 (free 0:128)
                    nc.tensor.matmul(ptile[0:64, 0:128], G_n, bdd[:, ls],
                                     start=True, stop=True)
                    nc.tensor.matmul(ptile[64:128, 0:128], G_n, bmb[:, ls],
                                     start=True, stop=True)
                    # SU -> partitions 0:64 (free 128:256)
                    nc.tensor.matmul(ptile[0:64, 128:256], G_n, bud[:, ls],
                                     start=True, stop=True)
                    sbw = spool.tile([128, 256], bf16, tag="sbw")
                    nc.scalar.copy(sbw[:, 0:128], ptile[:, 0:128])
                    nc.scalar.copy(sbw[0:64, 128:256], ptile[0:64, 128:256])
                    obs = qpool.tile([128, 512], f32, tag="po")
                    nc.tensor.matmul(obs[:], sbw[:, 0:128], prbA_t[:, 0:512],
                                     start=True, stop=False)
                    nc.tensor.matmul(obs[:], sbw[0:64, 128:256], prbB_t[:, 0:512],
                                     start=False, stop=True)
                    ob2 = rpool2.tile([128, 64], f32, tag="po2")
                    nc.tensor.matmul(ob2[:], sbw[:, 0:128], prbA_t[:, 512:576],
                                     start=True, stop=False)
                    nc.tensor.matmul(ob2[:], sbw[0:64, 128:256], prbB_t[:, 512:576],
                                     start=False, stop=True)
                    osb = opool.tile([128, 512], f32, tag="osb")
                    nc.scalar.copy(osb[:, 0:256], obs[:, 0:256])
                    nc.vector.tensor_scalar_add(osb[:, 256:512], obs[:, 256:512], 0.0)
                    rsb = lpool.tile([128, 64], f32, tag="rsb")
                    nc.vector.tensor_scalar_mul(
                        rsb[:], ob2[:],
                        scol_t[:, NT2 + n:NT2 + n + 1])
                    # DMA out
                    p_lo = 0 if n == 0 else 7
                    p_hi = min(127, 2046 - STRIDE * n)
                    npn = p_hi - p_lo + 1
                    sim_dst = bass.AP(
                        OUTD.tensor,
                        (b * ROWS + 9 * (STRIDE * n + p_lo) + 1) * 64,
                        [[9 * 64, npn], [1, 512]])
                    nc.sync.dma_start(sim_dst, osb[p_lo:p_hi + 1, :])
                    real_dst = bass.AP(
                        OUTD.tensor,
                        (b * ROWS + 9 * (STRIDE * n + p_lo + 1)) * 64,
                        [[9 * 64, npn], [1, 64]])
                    nc.sync.dma_start(real_dst, rsb[p_lo:p_hi + 1, :])
    nc.compile()
    return nc


_NC_CACHE = None


def _in_maps(inputs):
    import ml_dtypes
    times = np.ascontiguousarray(inputs["times"], np.float32)
    feats = np.ascontiguousarray(inputs["features"], np.float32)
    npm = inputs["non_pad_mask"].astype(np.float32)
    u = np.asarray(inputs["uniform_sample"], np.float32)
    W = np.ascontiguousarray(inputs["W"], np.float32)
    bias = np.ascontiguousarray(inputs["bias_param"], np.float32)

    band, prbA, prbB = _consts(W, bias, u)
    hostF, trow, udtrow, npmrow, scol = _host_prep(times, feats, npm)

    in_maps = []
    for c in range(NCORES):
        sl = slice(c * BPC, (c + 1) * BPC)
        in_maps.append({
            "f": np.ascontiguousarray(hostF[sl]),
            "trow": np.ascontiguousarray(trow[sl]),
            "udtrow": np.ascontiguousarray(udtrow[sl]),
            "npmrow": np.ascontiguousarray(npmrow[sl]),
            "scol": np.ascontiguousarray(scol[sl]),
            "band": band, "prbA": prbA, "prbB": prbB,
        })
    return in_maps


def kernel(**inputs):
    global _NC_CACHE
    from concourse.bass_utils import run_bass_kernel_spmd

    if _NC_CACHE is None:
        _NC_CACHE = _build_nc()
    nc = _NC_CACHE
    in_maps = _in_maps(inputs)
    res = run_bass_kernel_spmd(nc, in_maps, core_ids=list(range(NCORES)))
    out = np.concatenate([r["out"] for r in res.results], 0)
    return out.astype(np.float32)


# revision 6
# speedup vs baseline: 1.2391x; 1.0406x over previous
"""Trainium2 Bass kernel for ContinuousConv1DSim (gnn_message_passing).

Delta-band bf16 formulation (validated in numpy emulation, rel err ~3e-3):
  Overlapping l-tiles of 128 events at stride 121 (window Ks=9 fully inside
  a tile for lanes p>=7), so no halo accumulation chain.
  G = F * npm_j (pre-masked on host, bf16).
  Per batch, band operands [j, (n,p)] built from gpsimd partition-broadcast
  rows:
    BandD = (tloc_l - tloc_j) * band  (delta-weighted band; tloc centered
                                       per tile so bf16 stays accurate)
    BandU = udt_l * band              (udt = masked dt to next event)
  MM1 (PE, bf16): ptile[c2, p] accumulates [SD | SG] and [SU]:
    SD = G^T @ BandD, SG = G^T @ band, SU = G^T @ BandU
  MM2 (PE, bf16): obsim[p, 576] = [SD|SG]^T @ prbA + [SU]^T @ prbB
    9 col-blocks of 64: s=0..7 sim rows and a u=1 block.
    block s: SD@W^T + SG@bias + udt*u_s*(SG@W^T)  == sim row s (pre-mask)
    u1 block * nsh == real row for l+1 (right-padding makes npm monotone).
  npm_l output masking is a per-partition scale on the PSUM->SBUF copies.
  PE loop is software-pipelined: MM1 of tile n+1 issues before MM2 of n.

Pure data parallel: batch 32 -> 8 cores x 4. All params replicated.
"""

import numpy as np

B, L, C, O, S = 32, 2048, 64, 64, 8
NCORES = 8
BPC = B // NCORES            # 4 batches per core
STRIDE = 121                 # l-tile stride (128 - 7 overlap)
NT2 = 17                     # tiles per batch: 121*16 + 128 > 2048
EXT = NT2 * 128              # 2176
ROWS = (L - 1) * (S + 1) + 1  # 18424


def _consts(W, bias, u):
    import ml_dtypes
    n = np.arange(128)
    band = ((n[:, None] >= n[None, :] - 7) & (n[:, None] <= n[None, :]))
    band = band.astype(np.float32)
    WT = W.T.astype(np.float32)
    prbA = np.zeros((128, 576), np.float32)
    prbB = np.zeros((128, 576), np.float32)
    for k in range(9):
        sl = slice(k * 64, (k + 1) * 64)
        prbA[0:64, sl] = WT           # SD rows -> W^T
        prbA[64:128, sl] = bias       # SG rows -> bias
        prbB[64:128, sl] = (u[k] if k < 8 else 1.0) * WT  # SU rows
    bf = ml_dtypes.bfloat16
    return band.astype(bf), prbA.astype(bf), prbB.astype(bf)


def _host_prep(times, feats, npm):
    """Per-full-batch host tensors (numpy, cheap)."""
    import ml_dtypes
    bf = ml_dtypes.bfloat16
    IDX = (STRIDE * np.arange(NT2))[None, :] + np.arange(128)[:, None]  # [p, n]
    IDXf = IDX.T.reshape(-1)        # [(n, p)] flattened

    G = feats * npm[:, :, None]     # mask padded events
    G_ext = np.concatenate([G, np.zeros((B, 256, C), np.float32)], 1)
    t_ext = np.concatenate([times, np.repeat(times[:, -1:], 256, 1)], 1)
    npm_ext = np.concatenate([npm, np.zeros((B, 256), np.float32)], 1)
    udt = np.zeros((B, L + 256), np.float32)
    udt[:, :L - 1] = (times[:, 1:] - times[:, :-1]) * npm[:, :-1] * npm[:, 1:]

    cen = times[:, STRIDE * np.arange(NT2)]          # [B, n]
    trow = (t_ext[:, IDXf].reshape(B, NT2, 128)
            - cen[:, :, None]).reshape(B, EXT).astype(np.float32)
    udtrow = udt[:, IDXf].astype(bf)                 # [B, EXT]
    # column scalars [p, n]: tloc, nsh (npm at l+1), npm
    scol = np.empty((B, 128, 3 * NT2), np.float32)
    scol[:, :, :NT2] = trow.reshape(B, NT2, 128).transpose(0, 2, 1)
    scol[:, :, NT2:2 * NT2] = npm_ext[:, IDX + 1]
    scol[:, :, 2 * NT2:] = npm_ext[:, IDX]
    hostF = G_ext[:, IDX, :].reshape(B, 128, NT2 * C).astype(bf)  # [B,p,(n c)]
    return hostF, trow, udtrow, scol


def _build_nc():
    import concourse.bass as bass
    import concourse.bacc as bacc
    import concourse.mybir as mybir
    import concourse.tile as tile

    f32 = mybir.dt.float32
    bf16 = mybir.dt.bfloat16
    Copy = mybir.ActivationFunctionType.Copy
    mult = mybir.AluOpType.mult
    add = mybir.AluOpType.add
    sub = mybir.AluOpType.subtract

    nc = bacc.Bacc("TRN2", target_bir_lowering=False, debug=False,
                   num_devices=NCORES)

    FD = nc.dram_tensor("f", [BPC, 128, NT2 * C], bf16, kind="ExternalInput").ap()
    TRD = nc.dram_tensor("trow", [BPC, EXT], f32, kind="ExternalInput").ap()
    URD = nc.dram_tensor("udtrow", [BPC, EXT], bf16, kind="ExternalInput").ap()
    SCD = nc.dram_tensor("scol", [BPC, 128, 3 * NT2], f32, kind="ExternalInput").ap()
    BDD = nc.dram_tensor("band", [128, 128], bf16, kind="ExternalInput").ap()
    PAD = nc.dram_tensor("prbA", [128, 576], bf16, kind="ExternalInput").ap()
    PBD = nc.dram_tensor("prbB", [128, 576], bf16, kind="ExternalInput").ap()
    OUTD = nc.dram_tensor("out", [BPC, ROWS, O], f32, kind="ExternalOutput").ap()

    with tile.TileContext(nc) as tc:
        with (
            tc.tile_pool(name="const", bufs=1) as cpool,
            tc.tile_pool(name="rows", bufs=2) as rpool,
            tc.tile_pool(name="rep", bufs=2) as bpool,
            tc.tile_pool(name="bands", bufs=2) as dpool,
            tc.tile_pool(name="feat", bufs=2) as fpool,
            tc.tile_pool(name="sbw", bufs=4) as spool,
            tc.tile_pool(name="osb", bufs=3) as opool,
            tc.tile_pool(name="rsb", bufs=3) as lpool,
            tc.tile_pool(name="pt", bufs=4, space=bass.MemorySpace.PSUM) as ppool,
            tc.tile_pool(name="po", bufs=2, space=bass.MemorySpace.PSUM) as qpool,
            tc.tile_pool(name="po2", bufs=2, space=bass.MemorySpace.PSUM) as rpool2,
        ):
            band_t = cpool.tile([128, 128], bf16, tag="band")
            prbA_t = cpool.tile([128, 576], bf16, tag="prbA")
            prbB_t = cpool.tile([128, 576], bf16, tag="prbB")
            zrow = cpool.tile([1, 64], f32, tag="zrow")
            nc.sync.dma_start(band_t[:], BDD)
            nc.sync.dma_start(prbA_t[:], PAD)
            nc.sync.dma_start(prbB_t[:], PBD)
            nc.gpsimd.memset(zrow[:], 0.0)
            bandv = band_t[:].unsqueeze(1).broadcast_to([128, NT2, 128])

            for b in range(BPC):
                trow_t = rpool.tile([1, EXT], f32, tag="trow")
                urow_t = rpool.tile([1, EXT], bf16, tag="urow")
                scol_t = rpool.tile([128, 3 * NT2], f32, tag="scol")
                fsb = fpool.tile([128, NT2 * C], bf16, tag="f")
                nc.sync.dma_start(trow_t[:], TRD[b].unsqueeze(0))
                nc.sync.dma_start(urow_t[:], URD[b].unsqueeze(0))
                nc.sync.dma_start(scol_t[:], SCD[b])
                nc.sync.dma_start(fsb[:], FD[b])
                nc.sync.dma_start(OUTD[b, 0:1, :], zrow[:])  # real row l=0

                trep = bpool.tile([128, EXT], f32, tag="trep")
                urep = bpool.tile([128, EXT], bf16, tag="urep")
                nc.gpsimd.partition_broadcast(trep[:], trow_t[:])
                nc.gpsimd.partition_broadcast(urep[:], urow_t[:])

                tcolv = scol_t[:, 0:NT2].unsqueeze(2).broadcast_to([128, NT2, 128])
                dif = dpool.tile([128, EXT], bf16, tag="dif")
                bdd = dpool.tile([128, EXT], bf16, tag="bd")
                bud = dpool.tile([128, EXT], bf16, tag="bu")
                nc.vector.scalar_tensor_tensor(
                    dif[:].rearrange("p (n l) -> p n l", l=128),
                    trep[:].rearrange("p (n l) -> p n l", l=128), 0.0, tcolv,
                    op0=add, op1=sub)
                nc.vector.scalar_tensor_tensor(
                    bdd[:].rearrange("p (n l) -> p n l", l=128),
                    dif[:].rearrange("p (n l) -> p n l", l=128), 1.0, bandv,
                    op0=mult, op1=mult)
                nc.vector.scalar_tensor_tensor(
                    bud[:].rearrange("p (n l) -> p n l", l=128),
                    urep[:].rearrange("p (n l) -> p n l", l=128), 1.0, bandv,
                    op0=mult, op1=mult)

                # software-pipelined tile loop: MM1 of n+1 before MM2 of n
                def mm1(n):
                    ls = slice(n * 128, (n + 1) * 128)
                    G_n = fsb[:, n * C:(n + 1) * C]
                    ptile = ppool.tile([128, 256], f32, tag="pt")
                    # SD -> part 0:64 (free 0:128), SG -> part 64:128
                    nc.tensor.matmul(ptile[0:64, 0:128], G_n, bdd[:, ls],
                                     start=True, stop=True)
                    nc.tensor.matmul(ptile[64:128, 0:128], G_n, band_t[:],
                                     start=True, stop=True)
                    # SU -> part 64:128 (free 128:256)
                    nc.tensor.matmul(ptile[64:128, 128:256], G_n, bud[:, ls],
                                     start=True, stop=True)
                    sbw = spool.tile([128, 256], bf16, tag="sbw")
                    nc.scalar.copy(sbw[:], ptile[:])
                    return sbw

                sbw_cur = mm1(0)
                for n in range(NT2):
                    sbw_nxt = mm1(n + 1) if n + 1 < NT2 else None
                    sbw = sbw_cur
                    obs = qpool.tile([128, 512], f32, tag="po")
                    nc.tensor.matmul(obs[:], sbw[:, 0:128], prbA_t[:, 0:512],
                                     start=True, stop=False)
                    nc.tensor.matmul(obs[:], sbw[64:128, 128:256],
                                     prbB_t[64:128, 0:512],
                                     start=False, stop=True)
                    ob2 = rpool2.tile([128, 64], f32, tag="po2")
                    nc.tensor.matmul(ob2[:], sbw[:, 0:128], prbA_t[:, 512:576],
                                     start=True, stop=False)
                    nc.tensor.matmul(ob2[:], sbw[64:128, 128:256],
                                     prbB_t[64:128, 512:576],
                                     start=False, stop=True)
                    sbw_cur = sbw_nxt
                    # npm_l masking via per-partition scale on the copies
                    osb = opool.tile([128, 512], f32, tag="osb")
                    nc.scalar.activation(osb[:, 0:288], obs[:, 0:288], Copy,
                                         scale=scol_t[:, 2 * NT2 + n:2 * NT2 + n + 1])
                    nc.vector.tensor_scalar_mul(
                        osb[:, 288:512], obs[:, 288:512],
                        scol_t[:, 2 * NT2 + n:2 * NT2 + n + 1])
                    rsb = lpool.tile([128, 64], f32, tag="rsb")
                    nc.vector.tensor_scalar_mul(
                        rsb[:], ob2[:],
                        scol_t[:, NT2 + n:NT2 + n + 1])
                    # DMA out
                    p_lo = 0 if n == 0 else 7
                    p_hi = min(127, 2046 - STRIDE * n)
                    npn = p_hi - p_lo + 1
                    sim_dst = bass.AP(
                        OUTD.tensor,
                        (b * ROWS + 9 * (STRIDE * n + p_lo) + 1) * 64,
                        [[9 * 64, npn], [1, 512]])
                    nc.sync.dma_start(sim_dst, osb[p_lo:p_hi + 1, :])
                    real_dst = bass.AP(
                        OUTD.tensor,
                        (b * ROWS + 9 * (STRIDE * n + p_lo + 1)) * 64,
                        [[9 * 64, npn], [1, 64]])
                    nc.sync.dma_start(real_dst, rsb[p_lo:p_hi + 1, :])
    nc.compile()
    return nc


_NC_CACHE = None


def _in_maps(inputs):
    times = np.ascontiguousarray(inputs["times"], np.float32)
    feats = np.ascontiguousarray(inputs["features"], np.float32)
    npm = inputs["non_pad_mask"].astype(np.float32)
    u = np.asarray(inputs["uniform_sample"], np.float32)
    W = np.ascontiguousarray(inputs["W"], np.float32)
    bias = np.ascontiguousarray(inputs["bias_param"], np.float32)

    band, prbA, prbB = _consts(W, bias, u)
    hostF, trow, udtrow, scol = _host_prep(times, feats, npm)

    in_maps = []
    for c in range(NCORES):
        sl = slice(c * BPC, (c + 1) * BPC)
        in_maps.append({
            "f": np.ascontiguousarray(hostF[sl]),
            "trow": np.ascontiguousarray(trow[sl]),
            "udtrow": np.ascontiguousarray(udtrow[sl]),
            "scol": np.ascontiguousarray(scol[sl]),
            "band": band, "prbA": prbA, "prbB": prbB,
        })
    return in_maps


def kernel(**inputs):
    global _NC_CACHE
    from concourse.bass_utils import run_bass_kernel_spmd

    if _NC_CACHE is None:
        _NC_CACHE = _build_nc()
    nc = _NC_CACHE
    in_maps = _in_maps(inputs)
    res = run_bass_kernel_spmd(nc, in_maps, core_ids=list(range(NCORES)))
    out = np.concatenate([r["out"] for r in res.results], 0)
    return out.astype(np.float32)


# revision 8
# speedup vs baseline: 1.3299x; 1.0734x over previous
"""Trainium2 Bass kernel for ContinuousConv1DSim (gnn_message_passing).

Delta-band bf16 formulation (validated in numpy emulation, rel err ~3e-3):
  Overlapping l-tiles of 128 events at stride 121 (window Ks=9 fully inside
  a tile for lanes p>=7), so no halo accumulation chain.
  G = F * npm_j (pre-masked on host, bf16).
  Per batch, band operands [j, (n,p)] built from gpsimd partition-broadcast
  rows:
    BandD = (tloc_l - tloc_j) * band  (delta-weighted band; tloc centered
                                       per tile so bf16 stays accurate)
    BandU = udt_l * band              (udt = masked dt to next event)
  MM1 (PE, bf16): ptile[c2, p]: SD = G^T @ BandD -> partitions 0:64;
    one merged matmul streams [band | BandU_n] -> SG, SU at partitions
    64:128 (free 0:128 / 128:256).
  MM2 (PE, bf16): obs[p, 512] = [SD|SG]^T @ prbA + [SU]^T @ prbB
    8 col-blocks of 64: block s = SD@W^T + SG@bias + udt*u_s*(SG@W^T)
    == sim row s before npm_l masking (applied as scale on PSUM->SBUF copy).
  Real row for l+1 = nsh * (simbase + udt*A) obtained as a fixed linear
  combination of blocks s=0 and s=7 (coefficients folded into host scalars);
  valid because right-padding makes npm monotone.
  PE loop is software-pipelined: MM1 of tile n+1 issues before MM2 of n,
  and the next batch's DMA/broadcast/band-builds are interleaved into the
  current batch's tile loop.

Pure data parallel: batch 32 -> 8 cores x 4. All params replicated.
"""

import numpy as np

B, L, C, O, S = 32, 2048, 64, 64, 8
NCORES = 8
BPC = B // NCORES            # 4 batches per core
STRIDE = 121                 # l-tile stride (128 - 7 overlap)
NT2 = 17                     # tiles per batch: 121*16 + 128 > 2048
EXT = NT2 * 128              # 2176
ROWS = (L - 1) * (S + 1) + 1  # 18424


def _consts(W, bias, u):
    import ml_dtypes
    n = np.arange(128)
    band = ((n[:, None] >= n[None, :] - 7) & (n[:, None] <= n[None, :]))
    band = band.astype(np.float32)
    WT = W.T.astype(np.float32)
    prbA = np.zeros((128, 512), np.float32)
    prbB = np.zeros((128, 512), np.float32)
    for k in range(8):
        sl = slice(k * 64, (k + 1) * 64)
        prbA[0:64, sl] = WT           # SD rows -> W^T
        prbA[64:128, sl] = bias       # SG rows -> bias
        prbB[64:128, sl] = u[k] * WT  # SU rows
    bf = ml_dtypes.bfloat16
    return band.astype(bf), prbA.astype(bf), prbB.astype(bf)


def _host_prep(times, feats, npm, u):
    """Per-full-batch host tensors (numpy, cheap)."""
    import ml_dtypes
    bf = ml_dtypes.bfloat16
    IDX = (STRIDE * np.arange(NT2))[None, :] + np.arange(128)[:, None]  # [p, n]
    IDXf = IDX.T.reshape(-1)        # [(n, p)] flattened

    G = feats * npm[:, :, None]     # mask padded events
    G_ext = np.concatenate([G, np.zeros((B, 256, C), np.float32)], 1)
    t_ext = np.concatenate([times, np.repeat(times[:, -1:], 256, 1)], 1)
    npm_ext = np.concatenate([npm, np.zeros((B, 256), np.float32)], 1)
    udt = np.zeros((B, L + 256), np.float32)
    udt[:, :L - 1] = (times[:, 1:] - times[:, :-1]) * npm[:, :-1] * npm[:, 1:]

    cen = times[:, STRIDE * np.arange(NT2)]          # [B, n]
    trow = (t_ext[:, IDXf].reshape(B, NT2, 128)
            - cen[:, :, None]).reshape(B, EXT).astype(np.float32)
    udtrow = udt[:, IDXf].astype(bf)                 # [B, EXT]
    # real row from blocks s=0, s=7: simbase + udt*A =
    #   (1-lam)*b0 + lam*b7 with lam = (1-u0)/(u7-u0)
    lam = float((1.0 - u[0]) / (u[7] - u[0]))
    nsh = npm_ext[:, IDX + 1]
    # column scalars [p, n]: tloc, npm, nsh*(1-lam), nsh*lam
    scol = np.empty((B, 128, 4 * NT2), np.float32)
    scol[:, :, :NT2] = trow.reshape(B, NT2, 128).transpose(0, 2, 1)
    scol[:, :, NT2:2 * NT2] = npm_ext[:, IDX]
    scol[:, :, 2 * NT2:3 * NT2] = nsh * (1.0 - lam)
    scol[:, :, 3 * NT2:] = nsh * lam
    hostF = G_ext[:, IDX, :].reshape(B, 128, NT2 * C).astype(bf)  # [B,p,(n c)]
    return hostF, trow, udtrow, scol


def _build_nc():
    import concourse.bass as bass
    import concourse.bacc as bacc
    import concourse.mybir as mybir
    import concourse.tile as tile

    f32 = mybir.dt.float32
    bf16 = mybir.dt.bfloat16
    Copy = mybir.ActivationFunctionType.Copy
    mult = mybir.AluOpType.mult
    add = mybir.AluOpType.add
    sub = mybir.AluOpType.subtract

    nc = bacc.Bacc("TRN2", target_bir_lowering=False, debug=False,
                   num_devices=NCORES)

    FD = nc.dram_tensor("f", [BPC, 128, NT2 * C], bf16, kind="ExternalInput").ap()
    TRD = nc.dram_tensor("trow", [BPC, EXT], f32, kind="ExternalInput").ap()
    URD = nc.dram_tensor("udtrow", [BPC, EXT], bf16, kind="ExternalInput").ap()
    SCD = nc.dram_tensor("scol", [BPC, 128, 4 * NT2], f32, kind="ExternalInput").ap()
    BDD = nc.dram_tensor("band", [128, 128], bf16, kind="ExternalInput").ap()
    PAD = nc.dram_tensor("prbA", [128, 512], bf16, kind="ExternalInput").ap()
    PBD = nc.dram_tensor("prbB", [128, 512], bf16, kind="ExternalInput").ap()
    OUTD = nc.dram_tensor("out", [BPC, ROWS, O], f32, kind="ExternalOutput").ap()

    with tile.TileContext(nc) as tc:
        with (
            tc.tile_pool(name="const", bufs=1) as cpool,
            tc.tile_pool(name="rows", bufs=2) as rpool,
            tc.tile_pool(name="rep", bufs=2) as bpool,
            tc.tile_pool(name="bands", bufs=2) as dpool,
            tc.tile_pool(name="bigbu", bufs=2) as gpool,
            tc.tile_pool(name="feat", bufs=2) as fpool,
            tc.tile_pool(name="sbw", bufs=4) as spool,
            tc.tile_pool(name="osb", bufs=3) as opool,
            tc.tile_pool(name="rsb", bufs=3) as lpool,
            tc.tile_pool(name="rt", bufs=3) as tpool,
            tc.tile_pool(name="pt", bufs=4, space=bass.MemorySpace.PSUM) as ppool,
            tc.tile_pool(name="po", bufs=3, space=bass.MemorySpace.PSUM) as qpool,
        ):
            band_t = cpool.tile([128, 128], bf16, tag="band")
            prbA_t = cpool.tile([128, 512], bf16, tag="prbA")
            prbB_t = cpool.tile([128, 512], bf16, tag="prbB")
            zrow = cpool.tile([1, 64], f32, tag="zrow")
            nc.sync.dma_start(band_t[:], BDD)
            nc.sync.dma_start(prbA_t[:], PAD)
            nc.sync.dma_start(prbB_t[:], PBD)
            nc.gpsimd.memset(zrow[:], 0.0)
            bandv = band_t[:].unsqueeze(1).broadcast_to([128, NT2, 128])

            state = {}

            def prep(b, step):
                """Emit prep piece `step` for batch b; returns nothing."""
                st = state.setdefault(b, {})
                if step == 0:
                    st['trow'] = rpool.tile([1, EXT], f32, tag="trow", name="trow")
                    st['urow'] = rpool.tile([1, EXT], bf16, tag="urow", name="urow")
                    st['scol'] = rpool.tile([128, 4 * NT2], f32, tag="scol", name="scol")
                    st['fsb'] = fpool.tile([128, NT2 * C], bf16, tag="f", name="fsb")
                    nc.sync.dma_start(st['trow'][:], TRD[b].unsqueeze(0))
                    nc.sync.dma_start(st['urow'][:], URD[b].unsqueeze(0))
                    nc.sync.dma_start(st['scol'][:], SCD[b])
                    nc.sync.dma_start(st['fsb'][:], FD[b])
                    nc.sync.dma_start(OUTD[b, 0:1, :], zrow[:])
                elif step == 1:
                    st['trep'] = bpool.tile([128, EXT], f32, tag="trep", name="trep")
                    nc.gpsimd.partition_broadcast(st['trep'][:], st['trow'][:])
                elif step == 2:
                    st['urep'] = bpool.tile([128, EXT], bf16, tag="urep", name="urep")
                    nc.gpsimd.partition_broadcast(st['urep'][:], st['urow'][:])
                elif step == 3:
                    st['dif'] = dpool.tile([128, EXT], bf16, tag="dif", name="dif")
                    tcolv = st['scol'][:, 0:NT2].unsqueeze(2) \
                        .broadcast_to([128, NT2, 128])
                    nc.vector.scalar_tensor_tensor(
                        st['dif'][:].rearrange("p (n l) -> p n l", l=128),
                        st['trep'][:].rearrange("p (n l) -> p n l", l=128),
                        0.0, tcolv, op0=add, op1=sub)
                elif step == 4:
                    st['bdd'] = dpool.tile([128, EXT], bf16, tag="bd", name="bdd")
                    nc.vector.scalar_tensor_tensor(
                        st['bdd'][:].rearrange("p (n l) -> p n l", l=128),
                        st['dif'][:].rearrange("p (n l) -> p n l", l=128),
                        1.0, bandv, op0=mult, op1=mult)
                elif step == 5:
                    # interleaved [band | BandU_n] per tile: [128, (n, 256)]
                    st['bigbu'] = gpool.tile([128, NT2 * 256], bf16, tag="bigbu", name="bigbu")
                    bb = st['bigbu'][:].rearrange("p (n l) -> p n l", l=256)
                    nc.scalar.copy(bb[:, :, 0:128], bandv)
                    nc.vector.scalar_tensor_tensor(
                        bb[:, :, 128:256],
                        st['urep'][:].rearrange("p (n l) -> p n l", l=128),
                        1.0, bandv, op0=mult, op1=mult)

            def mm1(b, n):
                st = state[b]
                G_n = st['fsb'][:, n * C:(n + 1) * C]
                ptile = ppool.tile([128, 256], f32, tag="pt")
                # SD -> partitions 0:64 (free 0:128)
                nc.tensor.matmul(ptile[0:64, 0:128], G_n,
                                 st['bdd'][:, n * 128:(n + 1) * 128],
                                 start=True, stop=True)
                # [SG | SU] -> partitions 64:128 (free 0:256), one stream
                nc.tensor.matmul(ptile[64:128, 0:256], G_n,
                                 st['bigbu'][:, n * 256:(n + 1) * 256],
                                 start=True, stop=True)
                sbw = spool.tile([128, 256], bf16, tag="sbw")
                nc.scalar.copy(sbw[:], ptile[:])
                return sbw

            PREP_AT = {3: 0, 5: 1, 7: 2, 9: 3, 12: 4, 15: 5}

            for b in range(BPC):
                if b == 0:
                    for s in range(6):
                        prep(0, s)
                st = state[b]
                scol_t = st['scol']
                sbw_cur = mm1(b, 0)
                for n in range(NT2):
                    if b + 1 < BPC and n in PREP_AT:
                        prep(b + 1, PREP_AT[n])
                    sbw_nxt = mm1(b, n + 1) if n + 1 < NT2 else None
                    sbw = sbw_cur
                    obs = qpool.tile([128, 512], f32, tag="po")
                    nc.tensor.matmul(obs[:], sbw[:, 0:128], prbA_t[:],
                                     start=True, stop=False)
                    nc.tensor.matmul(obs[:], sbw[64:128, 128:256],
                                     prbB_t[64:128, :],
                                     start=False, stop=True)
                    sbw_cur = sbw_nxt
                    # npm_l masking via per-partition scale on the copies
                    osb = opool.tile([128, 512], f32, tag="osb")
                    nc.scalar.activation(osb[:, 0:384], obs[:, 0:384], Copy,
                                         scale=scol_t[:, NT2 + n:NT2 + n + 1])
                    nc.vector.tensor_scalar_mul(
                        osb[:, 384:512], obs[:, 384:512],
                        scol_t[:, NT2 + n:NT2 + n + 1])
                    # real row l+1 = nshl*b0 + nshr*b7
                    rt = tpool.tile([128, 64], f32, tag="rt")
                    nc.vector.tensor_scalar_mul(
                        rt[:], obs[:, 0:64],
                        scol_t[:, 2 * NT2 + n:2 * NT2 + n + 1])
                    rsb = lpool.tile([128, 64], f32, tag="rsb")
                    nc.vector.scalar_tensor_tensor(
                        rsb[:], obs[:, 448:512],
                        scol_t[:, 3 * NT2 + n:3 * NT2 + n + 1], rt[:],
                        op0=mult, op1=add)
                    # DMA out
                    p_lo = 0 if n == 0 else 7
                    p_hi = min(127, 2046 - STRIDE * n)
                    npn = p_hi - p_lo + 1
                    sim_dst = bass.AP(
                        OUTD.tensor,
                        (b * ROWS + 9 * (STRIDE * n + p_lo) + 1) * 64,
                        [[9 * 64, npn], [1, 512]])
                    nc.sync.dma_start(sim_dst, osb[p_lo:p_hi + 1, :])
                    real_dst = bass.AP(
                        OUTD.tensor,
                        (b * ROWS + 9 * (STRIDE * n + p_lo + 1)) * 64,
                        [[9 * 64, npn], [1, 64]])
                    nc.sync.dma_start(real_dst, rsb[p_lo:p_hi + 1, :])
                del state[b]
    nc.compile()
    return nc


_NC_CACHE = None


def _in_maps(inputs):
    times = np.ascontiguousarray(inputs["times"], np.float32)
    feats = np.ascontiguousarray(inputs["features"], np.float32)
    npm = inputs["non_pad_mask"].astype(np.float32)
    u = np.asarray(inputs["uniform_sample"], np.float32)
    W = np.ascontiguousarray(inputs["W"], np.float32)
    bias = np.ascontiguousarray(inputs["bias_param"], np.float32)

    band, prbA, prbB = _consts(W, bias, u)
    hostF, trow, udtrow, scol = _host_prep(times, feats, npm, u)

    in_maps = []
    for c in range(NCORES):
        sl = slice(c * BPC, (c + 1) * BPC)
        in_maps.append({
            "f": np.ascontiguousarray(hostF[sl]),
            "trow": np.ascontiguousarray(trow[sl]),
            "udtrow": np.ascontiguousarray(udtrow[sl]),
            "scol": np.ascontiguousarray(scol[sl]),
            "band": band, "prbA": prbA, "prbB": prbB,
        })
    return in_maps


def kernel(**inputs):
    global _NC_CACHE
    from concourse.bass_utils import run_bass_kernel_spmd

    if _NC_CACHE is None:
        _NC_CACHE = _build_nc()
    nc = _NC_CACHE
    in_maps = _in_maps(inputs)
    res = run_bass_kernel_spmd(nc, in_maps, core_ids=list(range(NCORES)))
    out = np.concatenate([r["out"] for r in res.results], 0)
    return out.astype(np.float32)


# revision 9
# speedup vs baseline: 1.4259x; 1.0722x over previous
"""Trainium2 Bass kernel for ContinuousConv1DSim (gnn_message_passing).

Delta-band bf16 formulation (validated in numpy emulation, rel err ~3e-3):
  Overlapping l-tiles of 128 events at stride 121 (window Ks=9 fully inside
  a tile for lanes p>=7), so no halo accumulation chain.
  G = F * npm_j (pre-masked on host, bf16).
  Per batch, band operands [j, (n,p)] built from gpsimd partition-broadcast
  rows:
    BandD = (tloc_l - tloc_j) * band  (delta-weighted band; tloc centered
                                       per tile so bf16 stays accurate)
    BandU = udt_l * band              (udt = masked dt to next event)
  MM1 (PE, bf16): ptile[c2, p]: SD = G^T @ BandD -> partitions 0:64;
    one merged matmul streams [band | BandU_n] -> SG, SU at partitions
    64:128 (free 0:128 / 128:256).
  MM2 (PE, bf16): obs[p, 512] = [SD|SG]^T @ prbA + [SU]^T @ prbB
    8 col-blocks of 64: block s = SD@W^T + SG@bias + udt*u_s*(SG@W^T)
    == sim row s before npm_l masking (applied as scale on PSUM->SBUF copy).
  Real row for l+1 = nsh * (simbase + udt*A) obtained as a fixed linear
  combination of blocks s=0 and s=7 (coefficients folded into host scalars);
  valid because right-padding makes npm monotone.
  PE loop is software-pipelined: MM1 of tile n+1 issues before MM2 of n,
  and the next batch's DMA/broadcast/band-builds are interleaved into the
  current batch's tile loop.

Pure data parallel: batch 32 -> 8 cores x 4. All params replicated.
"""

import numpy as np

B, L, C, O, S = 32, 2048, 64, 64, 8
NCORES = 8
BPC = B // NCORES            # 4 batches per core
STRIDE = 121                 # l-tile stride (128 - 7 overlap)
NT2 = 17                     # tiles per batch: 121*16 + 128 > 2048
EXT = NT2 * 128              # 2176
ROWS = (L - 1) * (S + 1) + 1  # 18424


def _consts(W, bias, u):
    import ml_dtypes
    n = np.arange(128)
    band = ((n[:, None] >= n[None, :] - 7) & (n[:, None] <= n[None, :]))
    band = band.astype(np.float32)
    WT = W.T.astype(np.float32)
    prbA = np.zeros((128, 512), np.float32)
    prbB = np.zeros((128, 512), np.float32)
    for k in range(8):
        sl = slice(k * 64, (k + 1) * 64)
        prbA[0:64, sl] = WT           # SD rows -> W^T
        prbA[64:128, sl] = bias       # SG rows -> bias
        prbB[64:128, sl] = u[k] * WT  # SU rows
    bf = ml_dtypes.bfloat16
    return band.astype(bf), prbA.astype(bf), prbB.astype(bf)


def _host_prep(times, feats, npm, u):
    """Per-full-batch host tensors (numpy, cheap)."""
    import ml_dtypes
    bf = ml_dtypes.bfloat16
    IDX = (STRIDE * np.arange(NT2))[None, :] + np.arange(128)[:, None]  # [p, n]
    IDXf = IDX.T.reshape(-1)        # [(n, p)] flattened

    G = feats * npm[:, :, None]     # mask padded events
    G_ext = np.concatenate([G, np.zeros((B, 256, C), np.float32)], 1)
    t_ext = np.concatenate([times, np.repeat(times[:, -1:], 256, 1)], 1)
    npm_ext = np.concatenate([npm, np.zeros((B, 256), np.float32)], 1)
    udt = np.zeros((B, L + 256), np.float32)
    udt[:, :L - 1] = (times[:, 1:] - times[:, :-1]) * npm[:, :-1] * npm[:, 1:]

    cen = times[:, STRIDE * np.arange(NT2)]          # [B, n]
    trow = (t_ext[:, IDXf].reshape(B, NT2, 128)
            - cen[:, :, None]).reshape(B, EXT).astype(np.float32)
    udtrow = udt[:, IDXf].astype(bf)                 # [B, EXT]
    # host-built delta band: bandD[b, j, (n, p)] = (tloc_p - tloc_j) * band
    n128 = np.arange(128)
    bandm = ((n128[:, None] >= n128[None, :] - 7)
             & (n128[:, None] <= n128[None, :])).astype(np.float32)
    tl = trow.reshape(B, NT2, 128)
    bdh = (tl[:, :, None, :] - tl[:, :, :, None]) * bandm[None, None]
    bdh = bdh.transpose(0, 2, 1, 3).reshape(B, 128, EXT).astype(bf)
    # real row from blocks s=0, s=7: simbase + udt*A =
    #   (1-lam)*b0 + lam*b7 with lam = (1-u0)/(u7-u0)
    lam = float((1.0 - u[0]) / (u[7] - u[0]))
    nsh = npm_ext[:, IDX + 1]
    # column scalars [p, n]: tloc, npm, nsh*(1-lam), nsh*lam
    scol = np.empty((B, 128, 4 * NT2), np.float32)
    scol[:, :, :NT2] = trow.reshape(B, NT2, 128).transpose(0, 2, 1)
    scol[:, :, NT2:2 * NT2] = npm_ext[:, IDX]
    scol[:, :, 2 * NT2:3 * NT2] = nsh * (1.0 - lam)
    scol[:, :, 3 * NT2:] = nsh * lam
    hostF = G_ext[:, IDX, :].reshape(B, 128, NT2 * C).astype(bf)  # [B,p,(n c)]
    return hostF, bdh, udtrow, scol


def _build_nc():
    import concourse.bass as bass
    import concourse.bacc as bacc
    import concourse.mybir as mybir
    import concourse.tile as tile

    f32 = mybir.dt.float32
    bf16 = mybir.dt.bfloat16
    Copy = mybir.ActivationFunctionType.Copy
    mult = mybir.AluOpType.mult
    add = mybir.AluOpType.add
    sub = mybir.AluOpType.subtract

    nc = bacc.Bacc("TRN2", target_bir_lowering=False, debug=False,
                   num_devices=NCORES)

    FD = nc.dram_tensor("f", [BPC, 128, NT2 * C], bf16, kind="ExternalInput").ap()
    BDH = nc.dram_tensor("bdh", [BPC, 128, EXT], bf16, kind="ExternalInput").ap()
    URD = nc.dram_tensor("udtrow", [BPC, EXT], bf16, kind="ExternalInput").ap()
    SCD = nc.dram_tensor("scol", [BPC, 128, 4 * NT2], f32, kind="ExternalInput").ap()
    BDD = nc.dram_tensor("band", [128, 128], bf16, kind="ExternalInput").ap()
    PAD = nc.dram_tensor("prbA", [128, 512], bf16, kind="ExternalInput").ap()
    PBD = nc.dram_tensor("prbB", [128, 512], bf16, kind="ExternalInput").ap()
    OUTD = nc.dram_tensor("out", [BPC, ROWS, O], f32, kind="ExternalOutput").ap()

    with tile.TileContext(nc) as tc:
        with (
            tc.tile_pool(name="const", bufs=1) as cpool,
            tc.tile_pool(name="rows", bufs=2) as rpool,
            tc.tile_pool(name="rep", bufs=2) as bpool,
            tc.tile_pool(name="bands", bufs=2) as dpool,
            tc.tile_pool(name="bigbu", bufs=2) as gpool,
            tc.tile_pool(name="feat", bufs=2) as fpool,
            tc.tile_pool(name="sbw", bufs=6) as spool,
            tc.tile_pool(name="osb", bufs=3) as opool,
            tc.tile_pool(name="rsb", bufs=3) as lpool,
            tc.tile_pool(name="rt", bufs=3) as tpool,
            tc.tile_pool(name="pt", bufs=5, space=bass.MemorySpace.PSUM) as ppool,
            tc.tile_pool(name="po", bufs=3, space=bass.MemorySpace.PSUM) as qpool,
        ):
            band_t = cpool.tile([128, 128], bf16, tag="band")
            prbA_t = cpool.tile([128, 512], bf16, tag="prbA")
            prbB_t = cpool.tile([128, 512], bf16, tag="prbB")
            zrow = cpool.tile([1, 64], f32, tag="zrow")
            nc.sync.dma_start(band_t[:], BDD)
            nc.sync.dma_start(prbA_t[:], PAD)
            nc.sync.dma_start(prbB_t[:], PBD)
            nc.gpsimd.memset(zrow[:], 0.0)
            bandv = band_t[:].unsqueeze(1).broadcast_to([128, NT2, 128])

            state = {}

            def prep(b, step):
                """Emit prep piece `step` for batch b; returns nothing."""
                st = state.setdefault(b, {})
                if step == 0:
                    st['urow'] = rpool.tile([1, EXT], bf16, tag="urow", name="urow")
                    st['scol'] = rpool.tile([128, 4 * NT2], f32, tag="scol", name="scol")
                    st['fsb'] = fpool.tile([128, NT2 * C], bf16, tag="f", name="fsb")
                    st['bdd'] = dpool.tile([128, EXT], bf16, tag="bd", name="bdd")
                    st['bigbu'] = gpool.tile([128, NT2 * 256], bf16, tag="bigbu", name="bigbu")
                    nc.sync.dma_start(st['urow'][:], URD[b].unsqueeze(0))
                    nc.sync.dma_start(st['scol'][:], SCD[b])
                    nc.sync.dma_start(st['fsb'][:], FD[b])
                    nc.sync.dma_start(st['bdd'][:], BDH[b])
                    nc.sync.dma_start(OUTD[b, 0:1, :], zrow[:])
                    bb = st['bigbu'][:].rearrange("p (n l) -> p n l", l=256)
                    nc.scalar.copy(bb[:, :, 0:128], bandv)
                elif step == 1:
                    st['urep'] = bpool.tile([128, EXT], bf16, tag="urep", name="urep")
                    nc.gpsimd.partition_broadcast(st['urep'][:], st['urow'][:])
                elif step == 2:
                    bb = st['bigbu'][:].rearrange("p (n l) -> p n l", l=256)
                    nc.vector.scalar_tensor_tensor(
                        bb[:, :, 128:256],
                        st['urep'][:].rearrange("p (n l) -> p n l", l=128),
                        1.0, bandv, op0=mult, op1=mult)

            def mm1(b, n):
                st = state[b]
                G_n = st['fsb'][:, n * C:(n + 1) * C]
                ptile = ppool.tile([128, 256], f32, tag="pt")
                # SD -> partitions 0:64 (free 0:128)
                nc.tensor.matmul(ptile[0:64, 0:128], G_n,
                                 st['bdd'][:, n * 128:(n + 1) * 128],
                                 start=True, stop=True)
                # [SG | SU] -> partitions 64:128 (free 0:256), one stream
                nc.tensor.matmul(ptile[64:128, 0:256], G_n,
                                 st['bigbu'][:, n * 256:(n + 1) * 256],
                                 start=True, stop=True)
                sbw = spool.tile([128, 256], bf16, tag="sbw")
                nc.scalar.copy(sbw[:], ptile[:])
                return sbw

            PREP_AT = {4: 0, 8: 1, 11: 2}

            for b in range(BPC):
                if b == 0:
                    for s in range(3):
                        prep(0, s)
                st = state[b]
                scol_t = st['scol']
                sbws = [mm1(b, 0), mm1(b, 1)]
                for n in range(NT2):
                    if b + 1 < BPC and n in PREP_AT:
                        prep(b + 1, PREP_AT[n])
                    if n + 2 < NT2:
                        sbws.append(mm1(b, n + 2))
                    sbw = sbws.pop(0)
                    obs = qpool.tile([128, 512], f32, tag="po")
                    nc.tensor.matmul(obs[:], sbw[:, 0:128], prbA_t[:],
                                     start=True, stop=False)
                    nc.tensor.matmul(obs[:], sbw[64:128, 128:256],
                                     prbB_t[64:128, :],
                                     start=False, stop=True)
                    # npm_l masking via per-partition scale on the copies
                    osb = opool.tile([128, 512], f32, tag="osb")
                    nc.scalar.activation(osb[:, 0:320], obs[:, 0:320], Copy,
                                         scale=scol_t[:, NT2 + n:NT2 + n + 1])
                    nc.vector.tensor_scalar_mul(
                        osb[:, 320:512], obs[:, 320:512],
                        scol_t[:, NT2 + n:NT2 + n + 1])
                    # real row l+1 = nshl*b0 + nshr*b7
                    rt = tpool.tile([128, 64], f32, tag="rt")
                    nc.vector.tensor_scalar_mul(
                        rt[:], obs[:, 0:64],
                        scol_t[:, 2 * NT2 + n:2 * NT2 + n + 1])
                    rsb = lpool.tile([128, 64], f32, tag="rsb")
                    nc.vector.scalar_tensor_tensor(
                        rsb[:], obs[:, 448:512],
                        scol_t[:, 3 * NT2 + n:3 * NT2 + n + 1], rt[:],
                        op0=mult, op1=add)
                    # DMA out
                    p_lo = 0 if n == 0 else 7
                    p_hi = min(127, 2046 - STRIDE * n)
                    npn = p_hi - p_lo + 1
                    sim_dst = bass.AP(
                        OUTD.tensor,
                        (b * ROWS + 9 * (STRIDE * n + p_lo) + 1) * 64,
                        [[9 * 64, npn], [1, 512]])
                    nc.sync.dma_start(sim_dst, osb[p_lo:p_hi + 1, :])
                    real_dst = bass.AP(
                        OUTD.tensor,
                        (b * ROWS + 9 * (STRIDE * n + p_lo + 1)) * 64,
                        [[9 * 64, npn], [1, 64]])
                    nc.sync.dma_start(real_dst, rsb[p_lo:p_hi + 1, :])
                del state[b]
    nc.compile()
    return nc


_NC_CACHE = None


def _in_maps(inputs):
    times = np.ascontiguousarray(inputs["times"], np.float32)
    feats = np.ascontiguousarray(inputs["features"], np.float32)
    npm = inputs["non_pad_mask"].astype(np.float32)
    u = np.asarray(inputs["uniform_sample"], np.float32)
    W = np.ascontiguousarray(inputs["W"], np.float32)
    bias = np.ascontiguousarray(inputs["bias_param"], np.float32)

    band, prbA, prbB = _consts(W, bias, u)
    hostF, bdh, udtrow, scol = _host_prep(times, feats, npm, u)

    in_maps = []
    for c in range(NCORES):
        sl = slice(c * BPC, (c + 1) * BPC)
        in_maps.append({
            "f": np.ascontiguousarray(hostF[sl]),
            "bdh": np.ascontiguousarray(bdh[sl]),
            "udtrow": np.ascontiguousarray(udtrow[sl]),
            "scol": np.ascontiguousarray(scol[sl]),
            "band": band, "prbA": prbA, "prbB": prbB,
        })
    return in_maps


def kernel(**inputs):
    global _NC_CACHE
    from concourse.bass_utils import run_bass_kernel_spmd

    if _NC_CACHE is None:
        _NC_CACHE = _build_nc()
    nc = _NC_CACHE
    in_maps = _in_maps(inputs)
    res = run_bass_kernel_spmd(nc, in_maps, core_ids=list(range(NCORES)))
    out = np.concatenate([r["out"] for r in res.results], 0)
    return out.astype(np.float32)


# revision 11
# speedup vs baseline: 1.5253x; 1.0697x over previous
"""Trainium2 Bass kernel for ContinuousConv1DSim (gnn_message_passing).

Delta-band bf16 formulation (validated in numpy emulation, rel err ~3e-3):
  Overlapping l-tiles of 128 events at stride 121 (window Ks=9 fully inside
  a tile for lanes p>=7), so no halo accumulation chain.
  G = F * npm_j (pre-masked on host, bf16).
  Per batch, band operands [j, (n,p)] built from gpsimd partition-broadcast
  rows:
    BandD = (tloc_l - tloc_j) * band  (delta-weighted band; tloc centered
                                       per tile so bf16 stays accurate)
    BandU = udt_l * band              (udt = masked dt to next event)
  MM1 (PE, bf16): ptile[c2, p]: SD = G^T @ BandD -> partitions 0:64;
    one merged matmul streams [band | BandU_n] -> SG, SU at partitions
    64:128 (free 0:128 / 128:256).
  MM2 (PE, bf16): obs[p, 512] = [SD|SG]^T @ prbA + [SU]^T @ prbB
    8 col-blocks of 64: block s = SD@W^T + SG@bias + udt*u_s*(SG@W^T)
    == sim row s before npm_l masking (applied as scale on PSUM->SBUF copy).
  Real row for l+1 = nsh * (simbase + udt*A) obtained as a fixed linear
  combination of blocks s=0 and s=7 (coefficients folded into host scalars);
  valid because right-padding makes npm monotone.
  PE loop is software-pipelined: MM1 of tile n+1 issues before MM2 of n,
  and the next batch's DMA/broadcast/band-builds are interleaved into the
  current batch's tile loop.

Pure data parallel: batch 32 -> 8 cores x 4. All params replicated.
"""

import numpy as np

B, L, C, O, S = 32, 2048, 64, 64, 8
NCORES = 8
BPC = B // NCORES            # 4 batches per core
STRIDE = 121                 # l-tile stride (128 - 7 overlap)
NT2 = 17                     # tiles per batch: 121*16 + 128 > 2048
EXT = NT2 * 128              # 2176
ROWS = (L - 1) * (S + 1) + 1  # 18424


def _consts(W, bias, u):
    import ml_dtypes
    n = np.arange(128)
    band = ((n[:, None] >= n[None, :] - 7) & (n[:, None] <= n[None, :]))
    band = band.astype(np.float32)
    WT = W.T.astype(np.float32)
    prbA = np.zeros((128, 512), np.float32)
    prbB = np.zeros((128, 512), np.float32)
    for k in range(8):
        sl = slice(k * 64, (k + 1) * 64)
        prbA[0:64, sl] = WT           # SD rows -> W^T
        prbA[64:128, sl] = bias       # SG rows -> bias
        prbB[64:128, sl] = u[k] * WT  # SU rows
    bf = ml_dtypes.bfloat16
    return band.astype(bf), prbA.astype(bf), prbB.astype(bf)


def _host_prep(times, feats, npm, u):
    """Per-full-batch host tensors (numpy, cheap)."""
    import ml_dtypes
    bf = ml_dtypes.bfloat16
    IDX = (STRIDE * np.arange(NT2))[None, :] + np.arange(128)[:, None]  # [p, n]
    IDXf = IDX.T.reshape(-1)        # [(n, p)] flattened

    G = feats * npm[:, :, None]     # mask padded events
    G_ext = np.concatenate([G, np.zeros((B, 256, C), np.float32)], 1)
    t_ext = np.concatenate([times, np.repeat(times[:, -1:], 256, 1)], 1)
    npm_ext = np.concatenate([npm, np.zeros((B, 256), np.float32)], 1)
    udt = np.zeros((B, L + 256), np.float32)
    udt[:, :L - 1] = (times[:, 1:] - times[:, :-1]) * npm[:, :-1] * npm[:, 1:]

    cen = times[:, STRIDE * np.arange(NT2)]          # [B, n]
    trow = (t_ext[:, IDXf].reshape(B, NT2, 128)
            - cen[:, :, None]).reshape(B, EXT).astype(np.float32)
    udtrow = udt[:, IDXf].astype(bf)                 # [B, EXT]
    # host-built delta band: bandD[b, j, (n, p)] = (tloc_p - tloc_j) * band
    n128 = np.arange(128)
    bandm = ((n128[:, None] >= n128[None, :] - 7)
             & (n128[:, None] <= n128[None, :])).astype(np.float32)
    tl = trow.reshape(B, NT2, 128)
    bdh = (tl[:, :, None, :] - tl[:, :, :, None]) * bandm[None, None]
    bdh = bdh.transpose(0, 2, 1, 3).reshape(B, 128, EXT).astype(bf)
    # real row from blocks s=0, s=7: simbase + udt*A =
    #   (1-lam)*b0 + lam*b7 with lam = (1-u0)/(u7-u0)
    lam = float((1.0 - u[0]) / (u[7] - u[0]))
    nsh = npm_ext[:, IDX + 1]
    # column scalars [p, n]: tloc, npm, nsh*(1-lam), nsh*lam
    scol = np.empty((B, 128, 4 * NT2), np.float32)
    scol[:, :, :NT2] = trow.reshape(B, NT2, 128).transpose(0, 2, 1)
    scol[:, :, NT2:2 * NT2] = npm_ext[:, IDX]
    scol[:, :, 2 * NT2:3 * NT2] = nsh * (1.0 - lam)
    scol[:, :, 3 * NT2:] = nsh * lam
    hostF = G_ext[:, IDX, :].reshape(B, 128, NT2 * C).astype(bf)  # [B,p,(n c)]
    return hostF, bdh, udtrow, scol


def _build_nc():
    import concourse.bass as bass
    import concourse.bacc as bacc
    import concourse.mybir as mybir
    import concourse.tile as tile

    f32 = mybir.dt.float32
    bf16 = mybir.dt.bfloat16
    Copy = mybir.ActivationFunctionType.Copy
    mult = mybir.AluOpType.mult
    add = mybir.AluOpType.add
    sub = mybir.AluOpType.subtract

    nc = bacc.Bacc("TRN2", target_bir_lowering=False, debug=False,
                   num_devices=NCORES)

    FD = nc.dram_tensor("f", [BPC, 128, NT2 * C], bf16, kind="ExternalInput").ap()
    BDH = nc.dram_tensor("bdh", [BPC, 128, EXT], bf16, kind="ExternalInput").ap()
    URD = nc.dram_tensor("udtrow", [BPC, EXT], bf16, kind="ExternalInput").ap()
    SCD = nc.dram_tensor("scol", [BPC, 128, 4 * NT2], f32, kind="ExternalInput").ap()
    BDD = nc.dram_tensor("band", [128, 128], bf16, kind="ExternalInput").ap()
    PAD = nc.dram_tensor("prbA", [128, 512], bf16, kind="ExternalInput").ap()
    PBD = nc.dram_tensor("prbB", [128, 512], bf16, kind="ExternalInput").ap()
    OUTD = nc.dram_tensor("out", [BPC, ROWS, O], f32, kind="ExternalOutput").ap()

    with tile.TileContext(nc) as tc:
        with (
            tc.tile_pool(name="const", bufs=1) as cpool,
            tc.tile_pool(name="rows", bufs=2) as rpool,
            tc.tile_pool(name="rep", bufs=2) as bpool,
            tc.tile_pool(name="bands", bufs=2) as dpool,
            tc.tile_pool(name="bigbu", bufs=2) as gpool,
            tc.tile_pool(name="feat", bufs=2) as fpool,
            tc.tile_pool(name="sbw", bufs=8) as spool,
            tc.tile_pool(name="osb", bufs=6) as opool,
            tc.tile_pool(name="rsb", bufs=6) as lpool,
            tc.tile_pool(name="rt", bufs=6) as tpool,
            tc.tile_pool(name="pt", bufs=5, space=bass.MemorySpace.PSUM) as ppool,
            tc.tile_pool(name="po", bufs=3, space=bass.MemorySpace.PSUM) as qpool,
        ):
            band_t = cpool.tile([128, 128], bf16, tag="band")
            prbA_t = cpool.tile([128, 512], bf16, tag="prbA")
            prbB_t = cpool.tile([128, 512], bf16, tag="prbB")
            zrow = cpool.tile([1, 64], f32, tag="zrow")
            nc.sync.dma_start(band_t[:], BDD)
            nc.sync.dma_start(prbA_t[:], PAD)
            nc.sync.dma_start(prbB_t[:], PBD)
            nc.gpsimd.memset(zrow[:], 0.0)
            bandv = band_t[:].unsqueeze(1).broadcast_to([128, NT2, 128])

            state = {}

            def prep(b, step):
                """Emit prep piece `step` for batch b; returns nothing."""
                st = state.setdefault(b, {})
                if step == 0:
                    st['urow'] = rpool.tile([1, EXT], bf16, tag="urow", name="urow")
                    st['scol'] = rpool.tile([128, 4 * NT2], f32, tag="scol", name="scol")
                    st['fsb'] = fpool.tile([128, NT2 * C], bf16, tag="f", name="fsb")
                    st['bdd'] = dpool.tile([128, EXT], bf16, tag="bd", name="bdd")
                    st['bigbu'] = gpool.tile([128, NT2 * 256], bf16, tag="bigbu", name="bigbu")
                    nc.sync.dma_start(st['urow'][:], URD[b].unsqueeze(0))
                    nc.sync.dma_start(st['scol'][:], SCD[b])
                    nc.sync.dma_start(st['fsb'][:], FD[b])
                    nc.sync.dma_start(st['bdd'][:], BDH[b])
                    nc.sync.dma_start(OUTD[b, 0:1, :], zrow[:])
                    bb = st['bigbu'][:].rearrange("p (n l) -> p n l", l=256)
                    nc.scalar.copy(bb[:, :, 0:128], bandv)
                elif step == 1:
                    st['urep'] = bpool.tile([128, EXT], bf16, tag="urep", name="urep")
                    nc.gpsimd.partition_broadcast(st['urep'][:], st['urow'][:])
                elif step == 2:
                    bb = st['bigbu'][:].rearrange("p (n l) -> p n l", l=256)
                    nc.vector.scalar_tensor_tensor(
                        bb[:, :, 128:256],
                        st['urep'][:].rearrange("p (n l) -> p n l", l=128),
                        1.0, bandv, op0=mult, op1=mult)

            def mm1(b, n):
                st = state[b]
                G_n = st['fsb'][:, n * C:(n + 1) * C]
                ptile = ppool.tile([128, 256], f32, tag="pt")
                # SD -> partitions 0:64 (free 0:128)
                nc.tensor.matmul(ptile[0:64, 0:128], G_n,
                                 st['bdd'][:, n * 128:(n + 1) * 128],
                                 start=True, stop=True)
                # [SG | SU] -> partitions 64:128 (free 0:256), one stream
                nc.tensor.matmul(ptile[64:128, 0:256], G_n,
                                 st['bigbu'][:, n * 256:(n + 1) * 256],
                                 start=True, stop=True)
                sbw = spool.tile([128, 256], bf16, tag="sbw")
                nc.scalar.copy(sbw[:], ptile[:])
                return sbw

            PREP_AT = {4: 0, 8: 1, 11: 2}

            for b in range(BPC):
                if b == 0:
                    for s in range(3):
                        prep(0, s)
                st = state[b]
                scol_t = st['scol']
                sbws = [mm1(b, 0), mm1(b, 1), mm1(b, 2)]
                for n in range(NT2):
                    if b + 1 < BPC and n in PREP_AT:
                        prep(b + 1, PREP_AT[n])
                    if n + 3 < NT2:
                        sbws.append(mm1(b, n + 3))
                    sbw = sbws.pop(0)
                    obs = qpool.tile([128, 512], f32, tag="po")
                    nc.tensor.matmul(obs[:], sbw[:, 0:128], prbA_t[:],
                                     start=True, stop=False)
                    nc.tensor.matmul(obs[:], sbw[64:128, 128:256],
                                     prbB_t[64:128, :],
                                     start=False, stop=True)
                    # npm_l masking via per-partition scale on the copies
                    osb = opool.tile([128, 512], f32, tag="osb")
                    nc.scalar.activation(osb[:, 0:320], obs[:, 0:320], Copy,
                                         scale=scol_t[:, NT2 + n:NT2 + n + 1])
                    nc.vector.tensor_scalar_mul(
                        osb[:, 320:512], obs[:, 320:512],
                        scol_t[:, NT2 + n:NT2 + n + 1])
                    # real row l+1 = nshl*b0 + nshr*b7
                    rt = tpool.tile([128, 64], f32, tag="rt")
                    nc.vector.tensor_scalar_mul(
                        rt[:], obs[:, 0:64],
                        scol_t[:, 2 * NT2 + n:2 * NT2 + n + 1])
                    rsb = lpool.tile([128, 64], f32, tag="rsb")
                    nc.vector.scalar_tensor_tensor(
                        rsb[:], obs[:, 448:512],
                        scol_t[:, 3 * NT2 + n:3 * NT2 + n + 1], rt[:],
                        op0=mult, op1=add)
                    # DMA out
                    p_lo = 0 if n == 0 else 7
                    p_hi = min(127, 2046 - STRIDE * n)
                    npn = p_hi - p_lo + 1
                    sim_dst = bass.AP(
                        OUTD.tensor,
                        (b * ROWS + 9 * (STRIDE * n + p_lo) + 1) * 64,
                        [[9 * 64, npn], [1, 512]])
                    nc.sync.dma_start(sim_dst, osb[p_lo:p_hi + 1, :])
                    real_dst = bass.AP(
                        OUTD.tensor,
                        (b * ROWS + 9 * (STRIDE * n + p_lo + 1)) * 64,
                        [[9 * 64, npn], [1, 64]])
                    nc.sync.dma_start(real_dst, rsb[p_lo:p_hi + 1, :])
                del state[b]
    nc.compile()
    return nc


_NC_CACHE = None


def _in_maps(inputs):
    times = np.ascontiguousarray(inputs["times"], np.float32)
    feats = np.ascontiguousarray(inputs["features"], np.float32)
    npm = inputs["non_pad_mask"].astype(np.float32)
    u = np.asarray(inputs["uniform_sample"], np.float32)
    W = np.ascontiguousarray(inputs["W"], np.float32)
    bias = np.ascontiguousarray(inputs["bias_param"], np.float32)

    band, prbA, prbB = _consts(W, bias, u)
    hostF, bdh, udtrow, scol = _host_prep(times, feats, npm, u)

    in_maps = []
    for c in range(NCORES):
        sl = slice(c * BPC, (c + 1) * BPC)
        in_maps.append({
            "f": np.ascontiguousarray(hostF[sl]),
            "bdh": np.ascontiguousarray(bdh[sl]),
            "udtrow": np.ascontiguousarray(udtrow[sl]),
            "scol": np.ascontiguousarray(scol[sl]),
            "band": band, "prbA": prbA, "prbB": prbB,
        })
    return in_maps


def kernel(**inputs):
    global _NC_CACHE
    from concourse.bass_utils import run_bass_kernel_spmd

    if _NC_CACHE is None:
        _NC_CACHE = _build_nc()
    nc = _NC_CACHE
    in_maps = _in_maps(inputs)
    res = run_bass_kernel_spmd(nc, in_maps, core_ids=list(range(NCORES)))
    out = np.concatenate([r["out"] for r in res.results], 0)
    return out.astype(np.float32)


# revision 12
# speedup vs baseline: 1.7036x; 1.1169x over previous
"""Trainium2 Bass kernel for ContinuousConv1DSim (gnn_message_passing).

Delta-band bf16 formulation (validated in numpy emulation, rel err ~3e-3):
  Overlapping l-tiles of 128 events at stride 121 (window Ks=9 fully inside
  a tile for lanes p>=7), so no halo accumulation chain.
  G = F * npm_j (pre-masked on host, bf16).
  Per batch, band operands [j, (n,p)] built from gpsimd partition-broadcast
  rows:
    BandD = (tloc_l - tloc_j) * band  (delta-weighted band; tloc centered
                                       per tile so bf16 stays accurate)
    BandU = udt_l * band              (udt = masked dt to next event)
  MM1 (PE, bf16): ptile[c2, p]: SD = G^T @ BandD -> partitions 0:64;
    one merged matmul streams [band | BandU_n] -> SG, SU at partitions
    64:128 (free 0:128 / 128:256).
  MM2 (PE, bf16): obs[p, 512] = [SD|SG]^T @ prbA + [SU]^T @ prbB
    8 col-blocks of 64: block s = SD@W^T + SG@bias + udt*u_s*(SG@W^T)
    == sim row s before npm_l masking (applied as scale on PSUM->SBUF copy).
  Real row for l+1 = nsh * (simbase + udt*A) obtained as a fixed linear
  combination of blocks s=0 and s=7 (coefficients folded into host scalars);
  valid because right-padding makes npm monotone.
  PE loop is software-pipelined: MM1 of tile n+1 issues before MM2 of n,
  and the next batch's DMA/broadcast/band-builds are interleaved into the
  current batch's tile loop.

Pure data parallel: batch 32 -> 8 cores x 4. All params replicated.
"""

import numpy as np

B, L, C, O, S = 32, 2048, 64, 64, 8
NCORES = 8
BPC = B // NCORES            # 4 batches per core
STRIDE = 121                 # l-tile stride (128 - 7 overlap)
NT2 = 17                     # tiles per batch: 121*16 + 128 > 2048
EXT = NT2 * 128              # 2176
ROWS = (L - 1) * (S + 1) + 1  # 18424


def _consts(W, bias, u):
    import ml_dtypes
    n = np.arange(128)
    band = ((n[:, None] >= n[None, :] - 7) & (n[:, None] <= n[None, :]))
    band = band.astype(np.float32)
    WT = W.T.astype(np.float32)
    prbA = np.zeros((128, 512), np.float32)
    prbB = np.zeros((128, 512), np.float32)
    for k in range(8):
        sl = slice(k * 64, (k + 1) * 64)
        prbA[0:64, sl] = WT           # SD rows -> W^T
        prbA[64:128, sl] = bias       # SG rows -> bias
        prbB[64:128, sl] = u[k] * WT  # SU rows
    bf = ml_dtypes.bfloat16
    return band.astype(bf), prbA.astype(bf), prbB.astype(bf)


def _host_prep(times, feats, npm, u):
    """Per-full-batch host tensors (numpy, cheap)."""
    import ml_dtypes
    bf = ml_dtypes.bfloat16
    IDX = (STRIDE * np.arange(NT2))[None, :] + np.arange(128)[:, None]  # [p, n]
    IDXf = IDX.T.reshape(-1)        # [(n, p)] flattened

    G = feats * npm[:, :, None]     # mask padded events
    G_ext = np.concatenate([G, np.zeros((B, 256, C), np.float32)], 1)
    t_ext = np.concatenate([times, np.repeat(times[:, -1:], 256, 1)], 1)
    npm_ext = np.concatenate([npm, np.zeros((B, 256), np.float32)], 1)
    udt = np.zeros((B, L + 256), np.float32)
    udt[:, :L - 1] = (times[:, 1:] - times[:, :-1]) * npm[:, :-1] * npm[:, 1:]

    cen = times[:, STRIDE * np.arange(NT2)]          # [B, n]
    trow = (t_ext[:, IDXf].reshape(B, NT2, 128)
            - cen[:, :, None]).reshape(B, EXT).astype(np.float32)
    udtrow = udt[:, IDXf].astype(bf)                 # [B, EXT]
    # host-built delta band: bandD[b, j, (n, p)] = (tloc_p - tloc_j) * band
    n128 = np.arange(128)
    bandm = ((n128[:, None] >= n128[None, :] - 7)
             & (n128[:, None] <= n128[None, :])).astype(np.float32)
    tl = trow.reshape(B, NT2, 128)
    bdh = (tl[:, :, None, :] - tl[:, :, :, None]) * bandm[None, None]
    bdh = bdh.transpose(0, 2, 1, 3).reshape(B, 128, EXT).astype(bf)
    # real row from blocks s=0, s=7: simbase + udt*A =
    #   (1-lam)*b0 + lam*b7 with lam = (1-u0)/(u7-u0)
    lam = float((1.0 - u[0]) / (u[7] - u[0]))
    nsh = npm_ext[:, IDX + 1]
    # column scalars [p, n]: tloc, npm, nsh*(1-lam), nsh*lam
    scol = np.empty((B, 128, 4 * NT2), np.float32)
    scol[:, :, :NT2] = trow.reshape(B, NT2, 128).transpose(0, 2, 1)
    scol[:, :, NT2:2 * NT2] = npm_ext[:, IDX]
    scol[:, :, 2 * NT2:3 * NT2] = nsh * (1.0 - lam)
    scol[:, :, 3 * NT2:] = nsh * lam
    hostF = G_ext[:, IDX, :].reshape(B, 128, NT2 * C).astype(bf)  # [B,p,(n c)]
    return hostF, bdh, udtrow, scol


def _build_nc():
    import concourse.bass as bass
    import concourse.bacc as bacc
    import concourse.mybir as mybir
    import concourse.tile as tile

    f32 = mybir.dt.float32
    bf16 = mybir.dt.bfloat16
    Copy = mybir.ActivationFunctionType.Copy
    mult = mybir.AluOpType.mult
    add = mybir.AluOpType.add
    sub = mybir.AluOpType.subtract

    nc = bacc.Bacc("TRN2", target_bir_lowering=False, debug=False,
                   num_devices=NCORES)

    FD = nc.dram_tensor("f", [BPC, 128, NT2 * C], bf16, kind="ExternalInput").ap()
    BDH = nc.dram_tensor("bdh", [BPC, 128, EXT], bf16, kind="ExternalInput").ap()
    URD = nc.dram_tensor("udtrow", [BPC, EXT], bf16, kind="ExternalInput").ap()
    SCD = nc.dram_tensor("scol", [BPC, 128, 4 * NT2], f32, kind="ExternalInput").ap()
    BDD = nc.dram_tensor("band", [128, 128], bf16, kind="ExternalInput").ap()
    PAD = nc.dram_tensor("prbA", [128, 512], bf16, kind="ExternalInput").ap()
    PBD = nc.dram_tensor("prbB", [128, 512], bf16, kind="ExternalInput").ap()
    OUTD = nc.dram_tensor("out", [BPC, ROWS, O], f32, kind="ExternalOutput").ap()

    with tile.TileContext(nc) as tc:
        with (
            tc.tile_pool(name="const", bufs=1) as cpool,
            tc.tile_pool(name="rows", bufs=2) as rpool,
            tc.tile_pool(name="rep", bufs=2) as bpool,
            tc.tile_pool(name="bands", bufs=2) as dpool,
            tc.tile_pool(name="bigbu", bufs=2) as gpool,
            tc.tile_pool(name="feat", bufs=2) as fpool,
            tc.tile_pool(name="sbw", bufs=8) as spool,
            tc.tile_pool(name="osb", bufs=6) as opool,
            tc.tile_pool(name="rsb", bufs=6) as lpool,
            tc.tile_pool(name="rt", bufs=6) as tpool,
            tc.tile_pool(name="pt", bufs=5, space=bass.MemorySpace.PSUM) as ppool,
            tc.tile_pool(name="po", bufs=3, space=bass.MemorySpace.PSUM) as qpool,
        ):
            band_t = cpool.tile([128, 128], bf16, tag="band")
            prbA_t = cpool.tile([128, 512], bf16, tag="prbA")
            prbB_t = cpool.tile([128, 512], bf16, tag="prbB")
            zrow = cpool.tile([1, 64], f32, tag="zrow")
            nc.sync.dma_start(band_t[:], BDD)
            nc.sync.dma_start(prbA_t[:], PAD)
            nc.sync.dma_start(prbB_t[:], PBD)
            nc.gpsimd.memset(zrow[:], 0.0)
            bandv = band_t[:].unsqueeze(1).broadcast_to([128, NT2, 128])

            state = {}

            def prep(b, step):
                """Emit prep piece `step` for batch b; returns nothing."""
                st = state.setdefault(b, {})
                if step == 0:
                    st['urow'] = rpool.tile([1, EXT], bf16, tag="urow", name="urow")
                    st['scol'] = rpool.tile([128, 4 * NT2], f32, tag="scol", name="scol")
                    st['fsb'] = fpool.tile([128, NT2 * C], bf16, tag="f", name="fsb")
                    st['bdd'] = dpool.tile([128, EXT], bf16, tag="bd", name="bdd")
                    st['bigbu'] = gpool.tile([128, NT2 * 256], bf16, tag="bigbu", name="bigbu")
                    nc.gpsimd.dma_start(st['urow'][:], URD[b].unsqueeze(0))
                    nc.gpsimd.dma_start(st['scol'][:], SCD[b])
                    nc.gpsimd.dma_start(st['fsb'][:], FD[b])
                    nc.gpsimd.dma_start(st['bdd'][:], BDH[b])
                    nc.gpsimd.dma_start(OUTD[b, 0:1, :], zrow[:])
                    bb = st['bigbu'][:].rearrange("p (n l) -> p n l", l=256)
                    nc.scalar.copy(bb[:, :, 0:128], bandv)
                elif step == 1:
                    st['urep'] = bpool.tile([128, EXT], bf16, tag="urep", name="urep")
                    nc.gpsimd.partition_broadcast(st['urep'][:], st['urow'][:])
                elif step == 2:
                    bb = st['bigbu'][:].rearrange("p (n l) -> p n l", l=256)
                    nc.vector.scalar_tensor_tensor(
                        bb[:, :, 128:256],
                        st['urep'][:].rearrange("p (n l) -> p n l", l=128),
                        1.0, bandv, op0=mult, op1=mult)

            def mm1(b, n):
                st = state[b]
                G_n = st['fsb'][:, n * C:(n + 1) * C]
                ptile = ppool.tile([128, 256], f32, tag="pt")
                # SD -> partitions 0:64 (free 0:128)
                nc.tensor.matmul(ptile[0:64, 0:128], G_n,
                                 st['bdd'][:, n * 128:(n + 1) * 128],
                                 start=True, stop=True)
                # [SG | SU] -> partitions 64:128 (free 0:256), one stream
                nc.tensor.matmul(ptile[64:128, 0:256], G_n,
                                 st['bigbu'][:, n * 256:(n + 1) * 256],
                                 start=True, stop=True)
                sbw = spool.tile([128, 256], bf16, tag="sbw")
                nc.scalar.copy(sbw[:], ptile[:])
                return sbw

            PREP_AT = {4: 0, 8: 1, 11: 2}

            for b in range(BPC):
                if b == 0:
                    for s in range(3):
                        prep(0, s)
                st = state[b]
                scol_t = st['scol']
                sbws = [mm1(b, 0), mm1(b, 1), mm1(b, 2)]
                for n in range(NT2):
                    if b + 1 < BPC and n in PREP_AT:
                        prep(b + 1, PREP_AT[n])
                    if n + 3 < NT2:
                        sbws.append(mm1(b, n + 3))
                    sbw = sbws.pop(0)
                    obs = qpool.tile([128, 512], f32, tag="po")
                    nc.tensor.matmul(obs[:], sbw[:, 0:128], prbA_t[:],
                                     start=True, stop=False)
                    nc.tensor.matmul(obs[:], sbw[64:128, 128:256],
                                     prbB_t[64:128, :],
                                     start=False, stop=True)
                    # npm_l masking via per-partition scale on the copies
                    osb = opool.tile([128, 512], f32, tag="osb")
                    nc.scalar.activation(osb[:, 0:320], obs[:, 0:320], Copy,
                                         scale=scol_t[:, NT2 + n:NT2 + n + 1])
                    nc.vector.tensor_scalar_mul(
                        osb[:, 320:512], obs[:, 320:512],
                        scol_t[:, NT2 + n:NT2 + n + 1])
                    # real row l+1 = nshl*b0 + nshr*b7
                    rt = tpool.tile([128, 64], f32, tag="rt")
                    nc.vector.tensor_scalar_mul(
                        rt[:], obs[:, 0:64],
                        scol_t[:, 2 * NT2 + n:2 * NT2 + n + 1])
                    rsb = lpool.tile([128, 64], f32, tag="rsb")
                    nc.vector.scalar_tensor_tensor(
                        rsb[:], obs[:, 448:512],
                        scol_t[:, 3 * NT2 + n:3 * NT2 + n + 1], rt[:],
                        op0=mult, op1=add)
                    # DMA out
                    p_lo = 0 if n == 0 else 7
                    p_hi = min(127, 2046 - STRIDE * n)
                    npn = p_hi - p_lo + 1
                    sim_dst = bass.AP(
                        OUTD.tensor,
                        (b * ROWS + 9 * (STRIDE * n + p_lo) + 1) * 64,
                        [[9 * 64, npn], [1, 512]])
                    nc.sync.dma_start(sim_dst, osb[p_lo:p_hi + 1, :])
                    real_dst = bass.AP(
                        OUTD.tensor,
                        (b * ROWS + 9 * (STRIDE * n + p_lo + 1)) * 64,
                        [[9 * 64, npn], [1, 64]])
                    nc.scalar.dma_start(real_dst, rsb[p_lo:p_hi + 1, :])
                del state[b]
    nc.compile()
    return nc


_NC_CACHE = None


def _in_maps(inputs):
    times = np.ascontiguousarray(inputs["times"], np.float32)
    feats = np.ascontiguousarray(inputs["features"], np.float32)
    npm = inputs["non_pad_mask"].astype(np.float32)
    u = np.asarray(inputs["uniform_sample"], np.float32)
    W = np.ascontiguousarray(inputs["W"], np.float32)
    bias = np.ascontiguousarray(inputs["bias_param"], np.float32)

    band, prbA, prbB = _consts(W, bias, u)
    hostF, bdh, udtrow, scol = _host_prep(times, feats, npm, u)

    in_maps = []
    for c in range(NCORES):
        sl = slice(c * BPC, (c + 1) * BPC)
        in_maps.append({
            "f": np.ascontiguousarray(hostF[sl]),
            "bdh": np.ascontiguousarray(bdh[sl]),
            "udtrow": np.ascontiguousarray(udtrow[sl]),
            "scol": np.ascontiguousarray(scol[sl]),
            "band": band, "prbA": prbA, "prbB": prbB,
        })
    return in_maps


def kernel(**inputs):
    global _NC_CACHE
    from concourse.bass_utils import run_bass_kernel_spmd

    if _NC_CACHE is None:
        _NC_CACHE = _build_nc()
    nc = _NC_CACHE
    in_maps = _in_maps(inputs)
    res = run_bass_kernel_spmd(nc, in_maps, core_ids=list(range(NCORES)))
    out = np.concatenate([r["out"] for r in res.results], 0)
    return out.astype(np.float32)
